# revision 1
# baseline (speedup 1.0000x reference)
"""Trainium2 Bass kernel for a pre-norm transformer block (MHSA + FFN).

Sharding: 8 cores, data parallel over (batch, seq-half). Core c handles
batch c//2, sequence half c%2. Inputs are permuted so each core's own
1024 tokens come first; attention K/V run over all 2048 tokens of the
batch (softmax is permutation invariant).

Matmul dtypes: f32r (TF32-like, ~1.5e-4 rel err) everywhere except the
FFN second half (h1/W2 in bf16). Softmax uses a constant exp shift
(logits are ~N(0, 26^2); exp(l - 128) stays inside fp32 range) and the
denominator is computed by a ones-column folded into the PV matmul,
normalized during the small o-transpose.
"""
import contextlib

import numpy as np
import ml_dtypes

import concourse.bass as bass
import concourse.tile as tile
import concourse.mybir as mybir
from concourse.bass_utils import run_bass_kernel_spmd
from concourse.masks import make_identity

B, T, C = 4, 2048, 1024
H, DH = 16, 64
DFF = 4 * C
N_CORES = 8
TQ = T // 2          # tokens owned per core
TS = T               # key/value tokens per core
NKO = C // 128       # 8 contraction tiles for C
F32R = mybir.dt.float32r
F32 = mybir.dt.float32
BF16 = mybir.dt.bfloat16
EXP_BIAS = -128.0
EPS = 1e-5

# ---------------------------------------------------------------------------
# Compat: this walrus build accepts at most 1 sem-wait per regular
# instruction (2 per InstEventSemaphore). bacc misses some tile-generated
# instructions, so split waits ourselves after finalize.
_ev_counter = [0]


def _legalize_sem_waits(nc):
    for func in nc.m.functions:
        for bb in func.blocks:
            new = []
            changed = False
            for inst in bb.instructions:
                si = inst.sync_info
                cap = 2 if isinstance(inst, mybir.InstEventSemaphore) else 1
                if si is not None and len(si.on_wait) > cap:
                    waits = list(si.on_wait)
                    for i in range(cap, len(waits), 2):
                        _ev_counter[0] += 1
                        e = mybir.InstEventSemaphore(
                            name=f"EVSPLIT-{_ev_counter[0]}", ins=[], outs=[])
                        e.engine = inst.engine
                        e.sync_info = mybir.SyncInfo(
                            on_wait=waits[i:i + 2], on_update=[])
                        new.append(e)
                    inst.sync_info = mybir.SyncInfo(
                        on_wait=waits[:cap], on_update=list(si.on_update))
                    changed = True
                new.append(inst)
            if changed:
                bb.instructions = new


# ---------------------------------------------------------------------------

def _layernorm_tile(nc, stats, work, x_ap, eps_t, out_ap):
    """LN over the free dim (1024) of x_ap [128, 1024] -> out_ap (any dtype)."""
    st = stats.tile([128, 2, 6], F32, tag="bnstats")
    mv = stats.tile([128, 2], F32, tag="bnaggr")
    xg = x_ap.rearrange("p (s d) -> p s d", s=2)
    for s in range(2):
        nc.vector.bn_stats(out=st[:, s, :], in_=xg[:, s, :])
    nc.vector.bn_aggr(out=mv[:], in_=st[:])
    rstd = stats.tile([128, 1], F32, tag="rstd")
    nc.scalar.activation(out=rstd[:], in_=mv[:, 1:2],
                         func=mybir.ActivationFunctionType.Sqrt,
                         bias=eps_t[:], scale=1.0)
    nc.vector.reciprocal(out=rstd[:], in_=rstd[:])
    nc.vector.tensor_scalar(out=out_ap, in0=x_ap,
                            scalar1=mv[:, 0:1], scalar2=rstd[:],
                            op0=mybir.AluOpType.subtract,
                            op1=mybir.AluOpType.mult)


def _build_nc():
    nc = bass.Bass()

    # ---- I/O ----
    x_d = nc.dram_tensor("x", [T, C], F32, kind="ExternalInput")
    wq_d = nc.dram_tensor("wq", [C, C], F32R, kind="ExternalInput")
    wk_d = nc.dram_tensor("wk", [C, C], F32R, kind="ExternalInput")
    wv_d = nc.dram_tensor("wv", [C, C], F32R, kind="ExternalInput")
    wo_d = nc.dram_tensor("wo", [C, C], F32R, kind="ExternalInput")
    w1_d = nc.dram_tensor("w1", [C, DFF], F32R, kind="ExternalInput")
    w2_d = nc.dram_tensor("w2", [DFF, C], BF16, kind="ExternalInput")
    bq_d = nc.dram_tensor("bq", [C], F32, kind="ExternalInput")
    bk_d = nc.dram_tensor("bk", [C], F32, kind="ExternalInput")
    bv_d = nc.dram_tensor("bv", [C], F32, kind="ExternalInput")
    bo_d = nc.dram_tensor("bo", [C], F32, kind="ExternalInput")
    b1_d = nc.dram_tensor("b1", [DFF], F32, kind="ExternalInput")
    b2_d = nc.dram_tensor("b2", [C], F32, kind="ExternalInput")
    ln1g_d = nc.dram_tensor("ln1g", [C], F32, kind="ExternalInput")
    ln1b_d = nc.dram_tensor("ln1b", [C], F32, kind="ExternalInput")
    ln2g_d = nc.dram_tensor("ln2g", [C], F32, kind="ExternalInput")
    ln2b_d = nc.dram_tensor("ln2b", [C], F32, kind="ExternalInput")
    out_d = nc.dram_tensor("out", [TQ, C], F32, kind="ExternalOutput")

    # ---- HBM scratch ----
    oT_h = nc.dram_tensor("oT_h", [NKO, 128, TQ], F32R)
    x2_h = nc.dram_tensor("x2_h", [TQ // 128, 128, C], F32)

    def bcast(ap, p=128):
        return bass.AP(tensor=ap.tensor, offset=ap.offset,
                       ap=[[0, p]] + [list(x) for x in ap.ap])

    with tile.TileContext(nc) as tc:
        with contextlib.ExitStack() as top:
            consts = top.enter_context(tc.tile_pool(name="consts", bufs=1))
            stats = top.enter_context(tc.tile_pool(name="stats", bufs=8))
            ps = top.enter_context(tc.tile_pool(name="ps", bufs=6, space="PSUM"))
            pst = top.enter_context(tc.tile_pool(name="pst", bufs=2, space="PSUM"))

            ident_f = consts.tile([128, 128], F32, tag="identf")
            make_identity(nc, ident_f)
            ident_r = consts.tile([128, 128], F32R, tag="identr")
            nc.vector.tensor_copy(out=ident_r[:], in_=ident_f[:])
            ebias = consts.tile([128, 1], F32, tag="ebias")
            nc.vector.memset(ebias[:], EXP_BIAS)
            eps_t = consts.tile([128, 1], F32, tag="eps")
            nc.vector.memset(eps_t[:], EPS)
            bq_s = consts.tile([128, NKO], F32, tag="bq")
            bk_s = consts.tile([128, NKO], F32, tag="bk")
            bo_s = consts.tile([128, NKO], F32, tag="bo")
            b2_s = consts.tile([128, NKO], F32, tag="b2")
            b1_s = consts.tile([128, DFF // 128], F32, tag="b1")
            for dst, src in ((bq_s, bq_d), (bk_s, bk_d), (bo_s, bo_d), (b2_s, b2_d), (b1_s, b1_d)):
                nc.sync.dma_start(out=dst[:], in_=src.rearrange("(o p) -> p o", p=128))
            bv_r = consts.tile([128, C], F32, tag="bvr")
            nc.gpsimd.dma_start(out=bv_r[:], in_=bcast(bv_d[:]))
            ln1g_s = consts.tile([128, NKO], F32, tag="ln1g")
            ln1b_s = consts.tile([128, NKO], F32, tag="ln1b")
            ln2g_s = consts.tile([128, NKO], F32, tag="ln2g")
            ln2b_s = consts.tile([128, NKO], F32, tag="ln2b")
            for dst, srct in ((ln1g_s, ln1g_d), (ln1b_s, ln1b_d), (ln2g_s, ln2g_d), (ln2b_s, ln2b_d)):
                nc.sync.dma_start(out=dst[:], in_=srct.rearrange("(o p) -> p o", p=128))

            # ============ Stages A-C: LN1, QKV, attention (interleaved) ====
            with contextlib.ExitStack() as abc:
                xnp = abc.enter_context(tc.tile_pool(name="xnp", bufs=1))
                xnT = xnp.tile([128, NKO, T], F32R, tag="xnT")

                # ---- Stage A: LN1 + transpose -> xnT ----
                with tc.tile_pool(name="workA", bufs=4) as workA:
                    for t in range(T // 128):
                        x_t = workA.tile([128, C], F32, tag="x_t")
                        nc.sync.dma_start(out=x_t[:], in_=x_d[t * 128:(t + 1) * 128, :])
                        xn_r = workA.tile([128, C], F32R, tag="xn_r")
                        _layernorm_tile(nc, stats, workA, x_t[:], eps_t, xn_r[:])
                        for c in range(NKO):
                            pt = pst.tile([128, 128], F32R, tag="pst")
                            nc.tensor.transpose(pt[:], xn_r[:, c * 128:(c + 1) * 128],
                                                ident_r[:])
                            nc.scalar.activation(out=xnT[:, c, t * 128:(t + 1) * 128],
                                                 in_=pt[:],
                                                 func=mybir.ActivationFunctionType.Identity,
                                                 bias=ln1b_s[:, c:c + 1],
                                                 scale=ln1g_s[:, c:c + 1])

                # ---- Stages B+C interleaved per group of 2 pairs ----
                wgp = abc.enter_context(tc.tile_pool(name="wgp", bufs=1))
                qkp = abc.enter_context(tc.tile_pool(name="qkp", bufs=2))
                vgp = abc.enter_context(tc.tile_pool(name="vgp", bufs=1))
                prb = abc.enter_context(tc.tile_pool(name="probs", bufs=1))
                opp = abc.enter_context(tc.tile_pool(name="opp", bufs=2))
                asm = abc.enter_context(tc.tile_pool(name="att_sm", bufs=3))

                wq_r = wq_d.rearrange("(o p) f -> p o f", p=128)
                wk_r = wk_d.rearrange("(o p) f -> p o f", p=128)
                wv_r = wv_d.rearrange("(o p) f -> p o f", p=128)

                qk_tiles = {}
                vg_tiles = {}
                PCH = 512  # probsT chunk width in tq

                def qkv_gen(g):
                    """Yield after each psum-group. Produces qk tiles for group g."""
                    wqt = wgp.tile([128, NKO, 256], F32R, tag="wqt")
                    wkt = wgp.tile([128, NKO, 256], F32R, tag="wkt")
                    nc.sync.dma_start(out=wqt[:], in_=wq_r[:, :, g * 256:(g + 1) * 256])
                    nc.sync.dma_start(out=wkt[:], in_=wk_r[:, :, g * 256:(g + 1) * 256])
                    for i, f in enumerate((2 * g, 2 * g + 1)):
                        qp = qkp.tile([128, TQ], F32R, tag=f"qp{i}")
                        kp = qkp.tile([128, TS], F32R, tag=f"kp{i}")
                        qk_tiles[2 * g + i] = (qp, kp)
                        for ch in range(TQ // 512):
                            pq = ps.tile([128, 512], F32, tag="ps")
                            for ko in range(NKO):
                                nc.tensor.matmul(pq[:], wqt[:, ko, i * 128:(i + 1) * 128],
                                                 xnT[:, ko, ch * 512:(ch + 1) * 512],
                                                 start=(ko == 0), stop=(ko == NKO - 1))
                            nc.scalar.activation(out=qp[:, ch * 512:(ch + 1) * 512], in_=pq[:],
                                                 func=mybir.ActivationFunctionType.Identity,
                                                 bias=bq_s[:, f:f + 1], scale=1.0)
                            yield
                        for ch in range(TS // 512):
                            pk = ps.tile([128, 512], F32, tag="ps")
                            for ko in range(NKO):
                                nc.tensor.matmul(pk[:], wkt[:, ko, i * 128:(i + 1) * 128],
                                                 xnT[:, ko, ch * 512:(ch + 1) * 512],
                                                 start=(ko == 0), stop=(ko == NKO - 1))
                            nc.scalar.activation(out=kp[:, ch * 512:(ch + 1) * 512], in_=pk[:],
                                                 func=mybir.ActivationFunctionType.Identity,
                                                 bias=bk_s[:, f:f + 1], scale=1.0)
                            yield
                def v_gen(g):
                    wvt = wgp.tile([128, NKO, 256], F32R, tag="wvt")
                    nc.sync.dma_start(out=wvt[:], in_=wv_r[:, :, g * 256:(g + 1) * 256])
                    vg = vgp.tile([128, TS // 128, 4, 65], F32R, tag="vg")
                    vg_tiles[g] = vg
                    nc.vector.memset(vg[:, :, :, DH:DH + 1].bitcast(F32), 1.0)
                    for to in range(TS // 128):
                        pv = ps.tile([128, 512], F32, tag="ps")
                        for ko in range(NKO):
                            nc.tensor.matmul(pv[0:128, 0:256], xnT[:, ko, to * 128:(to + 1) * 128],
                                             wvt[:, ko, :],
                                             start=(ko == 0), stop=(ko == NKO - 1))
                        nc.vector.tensor_add(
                            out=vg[:, to, :, 0:DH],
                            in0=pv[:, 0:256].rearrange("p (h d) -> p h d", d=DH),
                            in1=bv_r[:, g * 256:(g + 1) * 256].rearrange("p (h d) -> p h d", d=DH))
                        yield

                def attn_gen(pair):
                    """Yield after each (head, chunk) unit."""
                    g = pair // 2
                    qp, kp = qk_tiles[pair]
                    vg = vg_tiles[g]
                    opair = opp.tile([128, TQ // 128, 128], F32, tag="opair")
                    for h2 in range(2):
                        h = pair * 2 + h2
                        hl = h % 4
                        base = h2 * 64
                        for ch in range(TQ // PCH):
                            probsT = prb.tile([128, TS // 128, PCH], F32R, tag="probsT")
                            for tso in range(TS // 128):
                                sT = ps.tile([128, 512], F32, tag="ps")
                                nc.tensor.matmul(
                                    sT[:, 0:PCH], kp[base:base + DH, tso * 128:(tso + 1) * 128],
                                    qp[base:base + DH, ch * PCH:(ch + 1) * PCH],
                                    start=True, stop=True)
                                nc.scalar.activation(
                                    out=probsT[:, tso, :], in_=sT[:, 0:PCH],
                                    func=mybir.ActivationFunctionType.Exp,
                                    scale=8.0, bias=ebias[:])
                            ov = ps.tile([128, 512], F32, tag="ps")
                            for to in range(TS // 128):
                                nc.tensor.matmul(ov[0:DH + 1, 0:PCH], vg[:, to, hl, 0:DH + 1],
                                                 probsT[:, to, :],
                                                 start=(to == 0), stop=(to == TS // 128 - 1))
                            ouT = asm.tile([72, PCH], F32R, tag="ouT")
                            nc.vector.tensor_copy(out=ouT[0:DH + 1, :], in_=ov[0:DH + 1, 0:PCH])
                            for bb in range(PCH // 128):
                                tqi = ch * (PCH // 128) + bb
                                ot = pst.tile([128, 128], F32R, tag="pst")
                                nc.tensor.transpose(ot[:, 0:72],
                                                    ouT[:, bb * 128:(bb + 1) * 128],
                                                    ident_r[0:72, 0:72])
                                r = asm.tile([128, 1], F32, tag="recip")
                                nc.vector.reciprocal(
                                    out=r[:], in_=ot[:, DH:DH + 1].bitcast(F32))
                                nc.vector.tensor_scalar_mul(
                                    out=opair[:, tqi, base:base + DH],
                                    in0=ot[:, 0:DH].bitcast(F32), scalar1=r[:])
                            yield
                    for t in range(TQ // 128):
                        po = pst.tile([128, 128], F32, tag="pst")
                        nc.tensor.transpose(po[:], opair[:, t, :], ident_f[:])
                        st = asm.tile([128, 128], F32R, tag="ost")
                        nc.vector.tensor_copy(out=st[:], in_=po[:])
                        nc.sync.dma_start(out=oT_h[pair, :, t * 128:(t + 1) * 128], in_=st[:])
                    yield

                def drain(gen, n=None):
                    k = 0
                    for _ in gen:
                        k += 1
                        if n is not None and k >= n:
                            return True
                    return False

                # software pipeline: Q/K of group g+1 interleave with attention
                # of group g; V of group g+1 is emitted at the group boundary
                # (after the last PV read of vg(g), vgp bufs=1).
                drain(qkv_gen(0))
                drain(v_gen(0))
                cur = [None]
                nqk = [1]

                def pull_qk(pair, n):
                    for _ in range(n):
                        # group g touches qkp slot g%2 == slot of group g-2; only
                        # start it once attention has moved past group g-2.
                        if cur[0] is None and nqk[0] < 4 and nqk[0] <= pair // 2 + 1:
                            cur[0] = qkv_gen(nqk[0])
                            nqk[0] += 1
                        if cur[0] is None:
                            return
                        if not drain(cur[0], 1):
                            cur[0] = None

                for pair in range(H // 2):
                    a = attn_gen(pair)
                    while drain(a, 1):
                        pull_qk(pair, 2)
                    if pair % 2 == 1 and pair // 2 + 1 < 4:
                        drain(v_gen(pair // 2 + 1))

            # ============ Stage D: Wo + residual + LN2 ============
            with contextlib.ExitStack() as dstk:
                fm4 = dstk.enter_context(tc.tile_pool(name="fm4", bufs=1))
                xn2T = fm4.tile([128, NKO, TQ], F32R, tag="fm4")
                with tc.tile_pool(name="dres", bufs=1) as dres, \
                     tc.tile_pool(name="workD", bufs=3) as workD:
                    oT = dres.tile([128, NKO, TQ], F32R, tag="oT")
                    nc.sync.dma_start(out=oT[:], in_=oT_h.rearrange("o p f -> p o f"))
                    wo_s = dres.tile([128, NKO, C], F32R, tag="wo")
                    nc.sync.dma_start(out=wo_s[:], in_=wo_d.rearrange("(o p) f -> p o f", p=128))
                    aoT = dres.tile([128, NKO, TQ], F32, tag="aoT")

                    for f in range(NKO):
                        for ch in range(TQ // 512):
                            pa = ps.tile([128, 512], F32, tag="ps")
                            for ko in range(NKO):
                                nc.tensor.matmul(pa[:], wo_s[:, ko, f * 128:(f + 1) * 128],
                                                 oT[:, ko, ch * 512:(ch + 1) * 512],
                                                 start=(ko == 0), stop=(ko == NKO - 1))
                            nc.scalar.activation(out=aoT[:, f, ch * 512:(ch + 1) * 512],
                                                 in_=pa[:],
                                                 func=mybir.ActivationFunctionType.Identity,
                                                 bias=bo_s[:, f:f + 1], scale=1.0)
                    for t in range(TQ // 128):
                        x_t = workD.tile([128, C], F32, tag="x_t")
                        nc.sync.dma_start(out=x_t[:], in_=x_d[t * 128:(t + 1) * 128, :])
                        x2_t = workD.tile([128, C], F32, tag="x2_t")
                        for c in range(NKO):
                            pt = pst.tile([128, 128], F32, tag="pst")
                            nc.tensor.transpose(pt[:], aoT[:, c, t * 128:(t + 1) * 128],
                                                ident_f[:])
                            nc.vector.tensor_add(out=x2_t[:, c * 128:(c + 1) * 128],
                                                 in0=pt[:], in1=x_t[:, c * 128:(c + 1) * 128])
                        nc.sync.dma_start(out=x2_h[t], in_=x2_t[:])
                        xn2_r = workD.tile([128, C], F32R, tag="xn_r")
                        _layernorm_tile(nc, stats, workD, x2_t[:], eps_t, xn2_r[:])
                        for c in range(NKO):
                            pt = pst.tile([128, 128], F32R, tag="pst")
                            nc.tensor.transpose(pt[:], xn2_r[:, c * 128:(c + 1) * 128],
                                                ident_r[:])
                            nc.scalar.activation(out=xn2T[:, c, t * 128:(t + 1) * 128],
                                                 in_=pt[:],
                                                 func=mybir.ActivationFunctionType.Identity,
                                                 bias=ln2b_s[:, c:c + 1],
                                                 scale=ln2g_s[:, c:c + 1])

                # ============ Stage E: FFN up (W1, relu) ============
                arena = dstk.enter_context(tc.tile_pool(name="arena", bufs=1))
                h1T = arena.tile([128, DFF // 128, TQ], BF16, tag="arena")
                with tc.tile_pool(name="w1p", bufs=2) as w1p:
                    for blk in range(DFF // 512):
                        w1t = w1p.tile([128, NKO, 512], F32R, tag="w1t")
                        nc.sync.dma_start(
                            out=w1t[:],
                            in_=w1_d.rearrange("(o p) f -> p o f", p=128)[:, :, blk * 512:(blk + 1) * 512])
                        for fs in range(4):
                            f = blk * 4 + fs
                            for ch in range(TQ // 512):
                                ph = ps.tile([128, 512], F32, tag="ps")
                                for ko in range(NKO):
                                    nc.tensor.matmul(ph[:], w1t[:, ko, fs * 128:(fs + 1) * 128],
                                                     xn2T[:, ko, ch * 512:(ch + 1) * 512],
                                                     start=(ko == 0), stop=(ko == NKO - 1))
                                nc.scalar.activation(out=h1T[:, f, ch * 512:(ch + 1) * 512],
                                                     in_=ph[:],
                                                     func=mybir.ActivationFunctionType.Relu,
                                                     bias=b1_s[:, f:f + 1], scale=1.0)

                # ============ Stage F: FFN down (W2) + residual + out ============
                ffnT = fm4.tile([128, NKO, TQ], F32, tag="fm4")
                with tc.tile_pool(name="w2p", bufs=2) as w2p:
                    for f in range(NKO):
                        w2t = w2p.tile([128, DFF // 128, 128], BF16, tag="w2t")
                        nc.sync.dma_start(
                            out=w2t[:],
                            in_=w2_d.rearrange("(o p) f -> p o f", p=128)[:, :, f * 128:(f + 1) * 128])
                        for ch in range(TQ // 512):
                            po2 = ps.tile([128, 512], F32, tag="ps")
                            for ko in range(DFF // 128):
                                nc.tensor.matmul(po2[:], w2t[:, ko, :],
                                                 h1T[:, ko, ch * 512:(ch + 1) * 512],
                                                 start=(ko == 0), stop=(ko == DFF // 128 - 1))
                            nc.scalar.activation(out=ffnT[:, f, ch * 512:(ch + 1) * 512],
                                                 in_=po2[:],
                                                 func=mybir.ActivationFunctionType.Identity,
                                                 bias=b2_s[:, f:f + 1], scale=1.0)
                with tc.tile_pool(name="workF", bufs=2) as workF:
                    for t in range(TQ // 128):
                        x2_t = workF.tile([128, C], F32, tag="x2_t")
                        nc.sync.dma_start(out=x2_t[:], in_=x2_h[t])
                        out_t = workF.tile([128, C], F32, tag="out_t")
                        for c in range(NKO):
                            pt = pst.tile([128, 128], F32, tag="pst")
                            nc.tensor.transpose(pt[:], ffnT[:, c, t * 128:(t + 1) * 128],
                                                ident_f[:])
                            nc.vector.tensor_add(out=out_t[:, c * 128:(c + 1) * 128],
                                                 in0=pt[:], in1=x2_t[:, c * 128:(c + 1) * 128])
                        nc.sync.dma_start(out=out_d[t * 128:(t + 1) * 128, :], in_=out_t[:])

    nc.finalize()
    _legalize_sem_waits(nc)
    return nc


_NC_CACHE = None


def _get_nc():
    global _NC_CACHE
    if _NC_CACHE is None:
        _NC_CACHE = _build_nc()
    return _NC_CACHE


def _shard_inputs(inputs):
    x = np.asarray(inputs["x"], np.float32)
    wq = np.ascontiguousarray(np.transpose(np.asarray(inputs["Wq"], np.float32), (1, 0, 2)).reshape(C, C))
    wk = np.ascontiguousarray(np.transpose(np.asarray(inputs["Wk"], np.float32), (1, 0, 2)).reshape(C, C))
    wv = np.ascontiguousarray(np.transpose(np.asarray(inputs["Wv"], np.float32), (1, 0, 2)).reshape(C, C))
    wo = np.ascontiguousarray(np.asarray(inputs["Wo"], np.float32))
    w1 = np.ascontiguousarray(np.asarray(inputs["W1"], np.float32))
    w2 = np.asarray(inputs["W2"], np.float32).astype(ml_dtypes.bfloat16)
    shared = {
        "wq": wq, "wk": wk, "wv": wv, "wo": wo, "w1": w1, "w2": w2,
        "bq": np.asarray(inputs["bq"], np.float32).reshape(C),
        "bk": np.asarray(inputs["bk"], np.float32).reshape(C),
        "bv": np.asarray(inputs["bv"], np.float32).reshape(C),
        "bo": np.asarray(inputs["bo"], np.float32).reshape(C),
        "b1": np.asarray(inputs["b1"], np.float32).reshape(DFF),
        "b2": np.asarray(inputs["b2"], np.float32).reshape(C),
        "ln1g": np.asarray(inputs["ln1_g"], np.float32),
        "ln1b": np.asarray(inputs["ln1_b"], np.float32),
        "ln2g": np.asarray(inputs["ln2_g"], np.float32),
        "ln2b": np.asarray(inputs["ln2_b"], np.float32),
    }
    in_maps = []
    for c in range(N_CORES):
        b, half = c // 2, c % 2
        own = x[b, half * TQ:(half + 1) * TQ]
        other = x[b, (1 - half) * TQ:(2 - half) * TQ]
        x_perm = np.ascontiguousarray(np.concatenate([own, other], axis=0))
        in_maps.append(dict(shared, x=x_perm))
    return in_maps


def _run(inputs, **spmd_kwargs):
    nc = _get_nc()
    in_maps = _shard_inputs(inputs)
    res = run_bass_kernel_spmd(nc, in_maps, core_ids=list(range(N_CORES)), **spmd_kwargs)
    out = np.empty((B, T, C), np.float32)
    for c in range(N_CORES):
        b, half = c // 2, c % 2
        out[b, half * TQ:(half + 1) * TQ] = res.results[c]["out"]
    return out, res


def kernel(**inputs) -> np.ndarray:
    out, _ = _run(inputs)
    return out



# revision 2
# speedup vs baseline: 1.0028x; 1.0028x over previous
"""Trainium2 Bass kernel for a pre-norm transformer block (MHSA + FFN), v2.

Sharding: 8 cores, data parallel over (batch, seq-half). Core c handles
batch c//2, sequence half c%2; K/V run over the full 2048 tokens
(softmax is permutation invariant, own tokens permuted first).

v2 vs v1:
- LN1 tiles interleave with pair-0 QKV so the PE starts early.
- Attention: ACT does ONLY exp, batched N=1024 (one activation per two
  score psum banks), probs written bf16; Q/K/V psum drains on DVE.
- PV keeps the fused ones-column denominator; drained via DVE
  reciprocal + DMA partition-broadcast + DVE multiply (no per-head
  transposes). Odd heads reach partitions 64-127 via SBUF->SBUF DMA.
- oT stays in SBUF (bf16); Wo/W1 bf16 paired with bf16 rhs.
- Stage D residual/LN2 interleaves with Wo ch1 / FFN-up; stage F
  re-streams W2 per 512-token chunk so chunk-0 epilogue overlaps
  chunk-1 matmuls.
- PSUM: attention = sc(4 banks) + qkv(2) + pv(2); D/E/F = ps(4)+pst(4).
"""
import contextlib

import numpy as np
import ml_dtypes

import concourse.bass as bass
import concourse.tile as tile
import concourse.mybir as mybir
from concourse.bass_utils import run_bass_kernel_spmd
from concourse.masks import make_identity

B, T, C = 4, 2048, 1024
H, DH = 16, 64
DFF = 4 * C
N_CORES = 8
TQ = T // 2
TS = T
NKO = C // 128
F32R = mybir.dt.float32r
F32 = mybir.dt.float32
BF16 = mybir.dt.bfloat16
EXP_BIAS = -128.0
EPS = 1e-5
AF = mybir.ActivationFunctionType
ALU = mybir.AluOpType

_ev_counter = [0]


def _legalize_sem_waits(nc):
    """This walrus build accepts at most 1 sem-wait per regular
    instruction (2 per InstEventSemaphore); split the rest ourselves."""
    for func in nc.m.functions:
        for bb in func.blocks:
            new = []
            changed = False
            for inst in bb.instructions:
                si = inst.sync_info
                cap = 2 if isinstance(inst, mybir.InstEventSemaphore) else 1
                if si is not None and len(si.on_wait) > cap:
                    waits = list(si.on_wait)
                    for i in range(cap, len(waits), 2):
                        _ev_counter[0] += 1
                        e = mybir.InstEventSemaphore(
                            name=f"EVSPLIT-{_ev_counter[0]}", ins=[], outs=[])
                        e.engine = inst.engine
                        e.sync_info = mybir.SyncInfo(
                            on_wait=waits[i:i + 2], on_update=[])
                        new.append(e)
                    inst.sync_info = mybir.SyncInfo(
                        on_wait=waits[:cap], on_update=list(si.on_update))
                    changed = True
                new.append(inst)
            if changed:
                bb.instructions = new


def _bcast_row(ap, p):
    """Broadcast a [1, n] AP across p partitions (DMA source)."""
    return bass.AP(tensor=ap.tensor, offset=ap.offset,
                   ap=[[0, p]] + [list(x) for x in ap.ap[1:]])


def _bcast_dram(ap, p=128):
    return bass.AP(tensor=ap.tensor, offset=ap.offset,
                   ap=[[0, p]] + [list(x) for x in ap.ap])


def _build_nc():
    nc = bass.Bass()

    x_d = nc.dram_tensor("x", [T, C], F32, kind="ExternalInput")
    wq_d = nc.dram_tensor("wq", [C, C], F32R, kind="ExternalInput")
    wk_d = nc.dram_tensor("wk", [C, C], F32R, kind="ExternalInput")
    wv_d = nc.dram_tensor("wv", [C, C], F32R, kind="ExternalInput")
    wo_d = nc.dram_tensor("wo", [C, C], BF16, kind="ExternalInput")
    w1_d = nc.dram_tensor("w1", [C, DFF], BF16, kind="ExternalInput")
    w2_d = nc.dram_tensor("w2", [DFF, C], BF16, kind="ExternalInput")
    bq_d = nc.dram_tensor("bq", [C], F32, kind="ExternalInput")
    bk_d = nc.dram_tensor("bk", [C], F32, kind="ExternalInput")
    bv_d = nc.dram_tensor("bv", [C], F32, kind="ExternalInput")
    bo_d = nc.dram_tensor("bo", [C], F32, kind="ExternalInput")
    b1_d = nc.dram_tensor("b1", [DFF], F32, kind="ExternalInput")
    b2_d = nc.dram_tensor("b2", [C], F32, kind="ExternalInput")
    out_d = nc.dram_tensor("out", [TQ, C], F32, kind="ExternalOutput")

    dr_h = nc.dram_tensor("dr_h", [H, 2, 512], F32)

    wq_r = wq_d.rearrange("(o p) f -> p o f", p=128)
    wk_r = wk_d.rearrange("(o p) f -> p o f", p=128)
    wv_r = wv_d.rearrange("(o p) f -> p o f", p=128)
    wo_r = wo_d.rearrange("(o p) f -> p o f", p=128)
    w1_r = w1_d.rearrange("(o p) f -> p o f", p=128)
    w2_r = w2_d.rearrange("(o p) f -> p o f", p=128)

    with tile.TileContext(nc) as tc:
        with contextlib.ExitStack() as top:
            consts = top.enter_context(tc.tile_pool(name="consts", bufs=1))
            stats = top.enter_context(tc.tile_pool(name="stats", bufs=8))

            identF = consts.tile([128, 128], F32, tag="identf")
            make_identity(nc, identF)
            identR = consts.tile([128, 128], F32R, tag="identr")
            nc.vector.tensor_copy(out=identR[:], in_=identF[:])
            ebias = consts.tile([128, 1], F32, tag="ebias")
            nc.vector.memset(ebias[:], EXP_BIAS)
            eps_t = consts.tile([128, 1], F32, tag="eps")
            nc.vector.memset(eps_t[:], EPS)
            bq_s = consts.tile([128, NKO], F32, tag="bq")
            bk_s = consts.tile([128, NKO], F32, tag="bk")
            bo_s = consts.tile([128, NKO], F32, tag="bo")
            b2_s = consts.tile([128, NKO], F32, tag="b2")
            b1_s = consts.tile([128, DFF // 128], F32, tag="b1")
            for dst, src in ((bq_s, bq_d), (bk_s, bk_d), (bo_s, bo_d),
                             (b2_s, b2_d), (b1_s, b1_d)):
                nc.sync.dma_start(out=dst[:],
                                  in_=src.rearrange("(o p) -> p o", p=128))
            bv_r = consts.tile([128, C], F32, tag="bvr")
            nc.gpsimd.dma_start(out=bv_r[:], in_=_bcast_dram(bv_d[:]))

            def layernorm(x_ap, out_ap):
                st = stats.tile([128, 2, 6], F32, tag="bnstats")
                mv = stats.tile([128, 2], F32, tag="bnaggr")
                xg = x_ap.rearrange("p (s d) -> p s d", s=2)
                for s in range(2):
                    nc.vector.bn_stats(out=st[:, s, :], in_=xg[:, s, :])
                nc.vector.bn_aggr(out=mv[:], in_=st[:])
                rstd = stats.tile([128, 1], F32, tag="rstd")
                nc.scalar.activation(out=rstd[:], in_=mv[:, 1:2],
                                     func=AF.Sqrt, bias=eps_t[:], scale=1.0)
                nc.vector.reciprocal(out=rstd[:], in_=rstd[:])
                nc.gpsimd.tensor_scalar(out=out_ap, in0=x_ap,
                                        scalar1=mv[:, 0:1], scalar2=rstd[:],
                                        op0=ALU.subtract, op1=ALU.mult)

            # oT outlives the attention scope (consumed by Wo)
            oTp = top.enter_context(tc.tile_pool(name="oTp", bufs=1))
            oT = oTp.tile([128, NKO, TQ], BF16, tag="oT")

            # =========== Attention phase ===========
            with contextlib.ExitStack() as abc:
                qkp = abc.enter_context(tc.tile_pool(name="qkp", bufs=2))
                vgp = abc.enter_context(tc.tile_pool(name="vgp", bufs=2))
                prb = abc.enter_context(tc.tile_pool(name="prb", bufs=2))
                drp = abc.enter_context(tc.tile_pool(name="drp", bufs=2))
                scp = abc.enter_context(
                    tc.tile_pool(name="scp", bufs=2, space="PSUM"))
                qkvp = abc.enter_context(
                    tc.tile_pool(name="qkvp", bufs=2, space="PSUM"))
                pvp = abc.enter_context(
                    tc.tile_pool(name="pvp", bufs=2, space="PSUM"))

                qk_tiles = {}
                vg_tiles = {}

                qkvscope = contextlib.ExitStack()
                if True:
                    xnp = qkvscope.enter_context(
                        tc.tile_pool(name="xnp", bufs=1))
                    xnT = xnp.tile([128, NKO, T], F32R, tag="xnT")
                    workA = qkvscope.enter_context(
                        tc.tile_pool(name="workA", bufs=3))
                    wgp = qkvscope.enter_context(
                        tc.tile_pool(name="wgp", bufs=2))

                    def a_unit(t):
                        x_t = workA.tile([128, C], F32, tag="x_t")
                        if t < 2:
                            with tc.high_priority():
                                nc.scalar.dma_start(
                                    out=x_t[:],
                                    in_=x_d[t * 128:(t + 1) * 128, :])
                        else:
                            nc.scalar.dma_start(
                                out=x_t[:],
                                in_=x_d[t * 128:(t + 1) * 128, :])
                        xn_r = workA.tile([128, C], F32R, tag="xn_r")
                        layernorm(x_t[:], xn_r[:])
                        pt = scp.tile([128, NKO, 128], F32R, tag="sc",
                                      name="pt")
                        for c in range(NKO):
                            nc.tensor.transpose(
                                pt[:, c, :],
                                xn_r[:, c * 128:(c + 1) * 128], identR[:])
                        nc.scalar.activation(
                            out=xnT[:, :, t * 128:(t + 1) * 128],
                            in_=pt[:], func=AF.Identity,
                            bias=0.0, scale=1.0)

                    def qk_gen(p):
                        wqt = wgp.tile([128, NKO, 128], F32R, tag="wqt",
                                       bufs=1)
                        wkt = wgp.tile([128, NKO, 128], F32R, tag="wkt",
                                       bufs=1)
                        nc.sync.dma_start(
                            out=wqt[:], in_=wq_r[:, :, p * 128:(p + 1) * 128])
                        nc.sync.dma_start(
                            out=wkt[:], in_=wk_r[:, :, p * 128:(p + 1) * 128])
                        qp = qkp.tile([128, TQ], F32R, tag="qp")
                        kp = qkp.tile([128, TS], F32R, tag="kp")
                        qk_tiles[p] = (qp, kp)
                        yield

                        def qu(ch):
                            pq = qkvp.tile([128, 512], F32, tag="qkv",
                                           name="pq")
                            for ko in range(NKO):
                                nc.tensor.matmul(
                                    pq[:], wqt[:, ko, :],
                                    xnT[:, ko, ch * 512:(ch + 1) * 512],
                                    start=(ko == 0), stop=(ko == NKO - 1))
                            nc.vector.tensor_scalar_add(
                                out=qp[:, ch * 512:(ch + 1) * 512],
                                in0=pq[:], scalar1=bq_s[:, p:p + 1])

                        def ku(ch):
                            pk = qkvp.tile([128, 512], F32, tag="qkv",
                                           name="pk")
                            for ko in range(NKO):
                                nc.tensor.matmul(
                                    pk[:], wkt[:, ko, :],
                                    xnT[:, ko, ch * 512:(ch + 1) * 512],
                                    start=(ko == 0), stop=(ko == NKO - 1))
                            nc.vector.tensor_scalar_add(
                                out=kp[:, ch * 512:(ch + 1) * 512],
                                in0=pk[:], scalar1=bk_s[:, p:p + 1])

                        for kind, ch in (("q", 0), ("k", 0), ("q", 1),
                                         ("k", 1), ("k", 2), ("k", 3)):
                            qu(ch) if kind == "q" else ku(ch)
                            yield

                    def v_gen(g):
                        wvt = wgp.tile([128, NKO, 256], F32R, tag="wvt",
                                       bufs=1)
                        nc.sync.dma_start(
                            out=wvt[:], in_=wv_r[:, :, g * 256:(g + 1) * 256])
                        vg = vgp.tile([128, TS // 128, 4, 65], BF16, tag="vg")
                        vg_tiles[g] = vg
                        nc.vector.memset(vg[:, :, :, 64:65], 1.0)
                        yield
                        for j in range(TS // 256):
                            pvv = qkvp.tile([128, 512], F32, tag="qkv")
                            for i in range(2):
                                to = 2 * j + i
                                for ko in range(NKO):
                                    nc.tensor.matmul(
                                        pvv[0:128, i * 256:(i + 1) * 256],
                                        xnT[:, ko, to * 128:(to + 1) * 128],
                                        wvt[:, ko, :],
                                        start=(ko == 0), stop=(ko == NKO - 1))
                            for i in range(2):
                                to = 2 * j + i
                                nc.vector.tensor_tensor(
                                    out=vg[:, to, :, 0:64],
                                    in0=pvv[:, i * 256:(i + 1) * 256].rearrange(
                                        "p (h d) -> p h d", d=64),
                                    in1=bv_r[:, g * 256:(g + 1) * 256].rearrange(
                                        "p (h d) -> p h d", d=64),
                                    op=ALU.add)
                            yield

                    # ---- prologue: LN1 tiles interleaved with pair-0 QKV
                    q0 = qk_gen(0)
                    next(q0)                    # wq/wk DMAs in flight
                    v0 = v_gen(0)
                    next(v0)                    # wv DMA + ones memset
                    for t in range(4):
                        a_unit(t)
                    next(q0)                    # Q(0,0)
                    next(q0)                    # K(0,0)
                    for t in range(4, 8):
                        a_unit(t)
                    next(q0)                    # Q(0,1)
                    next(q0)                    # K(0,1)
                    next(v0)                    # V(0,0)
                    next(v0)                    # V(0,1)
                    for t in range(8, 12):
                        a_unit(t)
                    next(q0)                    # K(0,2)
                    next(v0)                    # V(0,2)
                    next(v0)                    # V(0,3)
                    for t in range(12, 16):
                        a_unit(t)
                    next(q0)                    # K(0,3)
                    for _ in v0:
                        pass
                    for _ in q0:
                        pass

                    # ---- pair loop with QKV pulled between attention units
                    pull_src = {
                        0: [qk_gen(1)],
                        1: [qk_gen(2), v_gen(1)],
                        2: [qk_gen(3)],
                        3: [qk_gen(4), v_gen(2)],
                        4: [qk_gen(5)],
                        5: [qk_gen(6), v_gen(3)],
                        6: [qk_gen(7)],
                        7: [],
                    }

                    def attention_hc(h, ch, pull, pending):
                        p = h // 2
                        base = (h % 2) * 64
                        hl = h % 4
                        qp, kp = qk_tiles[p]
                        vg = vg_tiles[h // 4]
                        probsT = prb.tile([128, TS // 128, 512], BF16,
                                          tag="probs")
                        pvt = pvp.tile([128, 512], F32, tag="pv")

                        def s_unit(g):
                            sct = scp.tile([128, 1024], F32, tag="sc")
                            for i in range(2):
                                tso = 2 * g + i
                                nc.tensor.matmul(
                                    sct[:, i * 512:(i + 1) * 512],
                                    kp[base:base + DH,
                                       tso * 128:(tso + 1) * 128],
                                    qp[base:base + DH,
                                       ch * 512:(ch + 1) * 512],
                                    start=True, stop=True)
                            nc.scalar.activation(
                                out=probsT[:, 2 * g:2 * g + 2, :], in_=sct[:],
                                func=AF.Exp, scale=8.0, bias=ebias[:])

                        def p_unit(g):
                            for i in range(2):
                                to = 2 * g + i
                                nc.tensor.matmul(
                                    pvt[0:DH + 1, :],
                                    vg[:, to, hl, 0:DH + 1],
                                    probsT[:, to, :],
                                    start=(to == 0), stop=(to == 15))

                        s_unit(0)
                        s_unit(1)
                        if pending:
                            pending[0]()
                            pending.clear()
                        for g in range(6):
                            pull()
                            p_unit(g)
                            s_unit(g + 2)
                        p_unit(6)
                        p_unit(7)
                        pull()

                        def drain():
                        # denominator: reciprocal of psum row 64 (stays on
                        # partition 64), then DMA-broadcast to partitions
                        # 0-63 (compute engines cannot shift partitions)
                            rc = drp.tile([128, 512], F32, tag="rbc",
                                          name="rc")
                            nc.vector.reciprocal(out=rc[64:65, :],
                                                 in_=pvt[64:65, :])
                            nc.sync.dma_start(out=dr_h[h, ch, :],
                                              in_=rc[64:65, :])
                            nc.sync.dma_start(
                                out=rc[0:64, :],
                                in_=_bcast_dram(dr_h[h, ch, :], 64))
                            rb = rc
                            if h % 2 == 0:
                                nc.vector.tensor_tensor(
                                    out=oT[0:64, p,
                                           ch * 512:(ch + 1) * 512],
                                    in0=pvt[0:64, :], in1=rb[0:64, :],
                                    op=ALU.mult)
                            else:
                                so = drp.tile([64, 512], BF16, tag="so",
                                              name="so")
                                nc.vector.tensor_tensor(
                                    out=so[0:64, :], in0=pvt[0:64, :],
                                    in1=rb[0:64, :], op=ALU.mult)
                                nc.sync.dma_start(
                                    out=oT[64:128, p,
                                           ch * 512:(ch + 1) * 512],
                                    in_=so[0:64, :])
                        pending.append(drain)

                    pending_drain = []
                    wo_tiles = {}
                    w1_tiles = {}
                    x_tiles = {}
                    x2_tiles = {}
                    for p in range(8):
                        gens = pull_src[p]

                        def pull():
                            while gens:
                                try:
                                    next(gens[0])
                                    return
                                except StopIteration:
                                    gens.pop(0)

                        for h in (2 * p, 2 * p + 1):
                            for ch in range(2):
                                attention_hc(h, ch, pull, pending_drain)
                        # generators must drain within their pair so the
                        # xnT/wgp scope can close after pair 6
                        if p >= 5:
                            pull()
                            while gens:
                                pull()
                        if p == 6:
                            # xnT/workA/wgp are dead: free them and pull
                            # the stage-D working set in under pair 7
                            qkvscope.close()
                            wop = top.enter_context(
                                tc.tile_pool(name="wop", bufs=2,
                                             side="right"))
                            w1p = top.enter_context(
                                tc.tile_pool(name="w1p", bufs=6,
                                             side="right"))
                            x2w = top.enter_context(
                                tc.tile_pool(name="x2w", bufs=4,
                                             side="right"))

                            def load_wo(k):
                                if k >= 16 or k in wo_tiles:
                                    return
                                f = k % NKO
                                wot = wop.tile([128, NKO, 128], BF16,
                                               tag="wo", name="wot")
                                nc.sync.dma_start(
                                    out=wot[:],
                                    in_=wo_r[:, :, f * 128:(f + 1) * 128])
                                wo_tiles[k] = wot

                            def load_w1(fb):
                                if fb >= DFF // 256 or fb in w1_tiles:
                                    return
                                w1t = w1p.tile([128, NKO, 256], BF16,
                                               tag="w1t", name="w1t")
                                nc.sync.dma_start(
                                    out=w1t[:],
                                    in_=w1_r[:, :, fb * 256:(fb + 1) * 256])
                                w1_tiles[fb] = w1t

                            def load_x(t):
                                x_t = x2w.tile([128, C], F32, tag="x_t",
                                               name="x_t")
                                nc.scalar.dma_start(
                                    out=x_t[:],
                                    in_=x_d[t * 128:(t + 1) * 128, :])
                                x_tiles[t] = x_t
                                x2_tiles[t] = x2w.tile(
                                    [128, C], F32, tag="x2_t",
                                    name="x2_t", bufs=8)

                            for k in range(4):
                                load_wo(k)
                            load_w1(0)
                            load_w1(1)
                            for t in range(8):
                                load_x(t)
                    if pending_drain:
                        pending_drain[0]()
                        pending_drain.clear()

            # =========== Stages D/E/F ===========
            with contextlib.ExitStack() as dstk:
                w2p = dstk.enter_context(tc.tile_pool(name="w2p", bufs=2))
                fm = dstk.enter_context(tc.tile_pool(name="fm", bufs=1))
                workD = dstk.enter_context(tc.tile_pool(name="workD", bufs=2))
                psD = dstk.enter_context(
                    tc.tile_pool(name="psD", bufs=4, space="PSUM"))
                pstD = dstk.enter_context(
                    tc.tile_pool(name="pstD", bufs=4, space="PSUM"))

                w2_tiles = {}

                def load_w2(f):
                    if f >= NKO or f in w2_tiles:
                        return
                    w2t = w2p.tile([128, DFF // 128, 128], BF16, tag="w2t")
                    nc.sync.dma_start(
                        out=w2t[:], in_=w2_r[:, :, f * 128:(f + 1) * 128])
                    w2_tiles[f] = w2t

                xn2T = fm.tile([128, NKO, TQ], BF16, tag="xn2T")
                h1T = fm.tile([128, DFF // 128, TQ], BF16, tag="h1T")

                def wo_unit(f, ch):
                    k = ch * NKO + f
                    load_wo(k + 2)
                    ps = psD.tile([128, 512], F32, tag="ps")
                    for ko in range(NKO):
                        nc.tensor.matmul(
                            ps[:], wo_tiles[k][:, ko, :],
                            oT[:, ko, ch * 512:(ch + 1) * 512],
                            start=(ko == 0), stop=(ko == NKO - 1))
                    s = workD.tile([128, 512], F32R, tag="sfc", bufs=3)
                    nc.scalar.activation(out=s[:], in_=ps[:],
                                         func=AF.Identity,
                                         bias=bo_s[:, f:f + 1], scale=1.0)
                    pt = pstD.tile([128, 4, 128], F32R, tag="pst")
                    for tt in range(4):
                        t = ch * 4 + tt
                        nc.tensor.transpose(pt[:, tt, :],
                                            s[:, tt * 128:(tt + 1) * 128],
                                            identR[:])
                        nc.vector.tensor_tensor(
                            out=x2_tiles[t][:, f * 128:(f + 1) * 128],
                            in0=pt[:, tt, :],
                            in1=x_tiles[t][:, f * 128:(f + 1) * 128],
                            op=ALU.add)

                def d_finish(t):
                    x2_t = x2_tiles[t]
                    x_tiles.pop(t)
                    xn2_r = workD.tile([128, C], F32R, tag="xn2_r",
                                       bufs=1)
                    layernorm(x2_t[:], xn2_r[:])
                    for half in range(2):
                        pt = pstD.tile([128, 4, 128], F32R, tag="pst")
                        for j in range(4):
                            c = half * 4 + j
                            nc.tensor.transpose(
                                pt[:, j, :], xn2_r[:, c * 128:(c + 1) * 128],
                                identR[:])
                        nc.scalar.activation(
                            out=xn2T[:, half * 4:(half + 1) * 4,
                                     t * 128:(t + 1) * 128],
                            in_=pt[:], func=AF.Identity,
                            bias=0.0, scale=1.0)

                def e_unit(fb, fs, ch):
                    f = fb * 2 + fs
                    ps = psD.tile([128, 512], F32, tag="ps")
                    for ko in range(NKO):
                        nc.tensor.matmul(
                            ps[:], w1_tiles[fb][:, ko, fs * 128:(fs + 1) * 128],
                            xn2T[:, ko, ch * 512:(ch + 1) * 512],
                            start=(ko == 0), stop=(ko == NKO - 1))
                    nc.scalar.activation(out=h1T[:, f, ch * 512:(ch + 1) * 512],
                                         in_=ps[:], func=AF.Relu,
                                         bias=b1_s[:, f:f + 1], scale=1.0)

                def f_unit(f, ch, finish=False):
                    load_w2(f + 2)
                    ps = psD.tile([128, 512], F32, tag="ps")
                    for ko in range(DFF // 128):
                        nc.tensor.matmul(
                            ps[:], w2_tiles[f][:, ko, :],
                            h1T[:, ko, ch * 512:(ch + 1) * 512],
                            start=(ko == 0), stop=(ko == DFF // 128 - 1))
                    s = workD.tile([128, 512], F32R, tag="sfc", bufs=3)
                    nc.scalar.activation(out=s[:], in_=ps[:],
                                         func=AF.Identity,
                                         bias=b2_s[:, f:f + 1], scale=1.0)
                    pt = pstD.tile([128, 4, 128], F32R, tag="pst")
                    for tt in range(4):
                        t = ch * 4 + tt
                        nc.tensor.transpose(pt[:, tt, :],
                                            s[:, tt * 128:(tt + 1) * 128],
                                            identR[:])
                        # residual added in place onto the resident x2 tile
                        nc.vector.tensor_tensor(
                            out=x2_tiles[t][:, f * 128:(f + 1) * 128],
                            in0=pt[:, tt, :],
                            in1=x2_tiles[t][:, f * 128:(f + 1) * 128],
                            op=ALU.add)
                        if finish:
                            f_finish(t)

                def f_finish(t):
                    out_t = x2_tiles.pop(t)
                    eng = nc.gpsimd if t % 2 == 0 else nc.sync
                    eng.dma_start(out=out_d[t * 128:(t + 1) * 128, :],
                                  in_=out_t[:])

                # ---- Stage D: Wo + residual + LN2 ----
                for f in range(NKO):
                    wo_unit(f, 0)
                # ch1 Wo interleaved with ch0 LN2 finish; d_finish(t) must
                # precede wo_unit(t, 1) so the x2w buffer-rotation WAR sems
                # point backwards in each engine's stream (no deadlock)
                for f in range(NKO):
                    if f < 4:
                        d_finish(f)
                    wo_unit(f, 1)
                # ---- Stage E: phase 1 = fb0-3 ch0 only, interleaved with
                # the ch1 LN2 finishes; phase 2 = fb0-3 ch1; phase 3 = rest.
                # (w1p bufs=6 keeps fb0-3 alive until phase 2.) ----
                for fb in range(4):
                    load_w1(fb + 2)
                    e_unit(fb, 0, 0)
                    e_unit(fb, 1, 0)
                    d_finish(4 + fb)
                for fb in range(4):
                    load_w1(6 + fb)
                    e_unit(fb, 0, 1)
                    e_unit(fb, 1, 1)
                for fb in range(4, DFF // 256):
                    load_w1(fb + 2)
                    for fs in range(2):
                        for ch in range(2):
                            e_unit(fb, fs, ch)
                load_w2(0)
                load_w2(1)
                # ---- Stage F: f-outer, W2 loaded once, residual in place
                for f in range(NKO):
                    f_unit(f, 0, finish=(f == NKO - 1))
                    f_unit(f, 1, finish=(f == NKO - 1))

    nc.finalize()
    _legalize_sem_waits(nc)
    return nc


_NC_CACHE = None


def _get_nc():
    global _NC_CACHE
    if _NC_CACHE is None:
        _NC_CACHE = _build_nc()
    return _NC_CACHE


def _shard_inputs(inputs):
    x = np.asarray(inputs["x"], np.float32)
    # Fold the LN affine (g, b) into the following linear layers:
    #   (z*g + b) @ W + c  ==  z @ (diag(g) W) + (c + b @ W)
    g1 = np.asarray(inputs["ln1_g"], np.float32).reshape(C)
    b1n = np.asarray(inputs["ln1_b"], np.float32).reshape(C)
    g2 = np.asarray(inputs["ln2_g"], np.float32).reshape(C)
    b2n = np.asarray(inputs["ln2_b"], np.float32).reshape(C)
    wq = np.ascontiguousarray(
        np.transpose(np.asarray(inputs["Wq"], np.float32), (1, 0, 2)).reshape(C, C))
    wk = np.ascontiguousarray(
        np.transpose(np.asarray(inputs["Wk"], np.float32), (1, 0, 2)).reshape(C, C))
    wv = np.ascontiguousarray(
        np.transpose(np.asarray(inputs["Wv"], np.float32), (1, 0, 2)).reshape(C, C))
    bq = np.asarray(inputs["bq"], np.float32).reshape(C) + b1n @ wq
    bk = np.asarray(inputs["bk"], np.float32).reshape(C) + b1n @ wk
    bv = np.asarray(inputs["bv"], np.float32).reshape(C) + b1n @ wv
    wq = np.ascontiguousarray(wq * g1[:, None])
    wk = np.ascontiguousarray(wk * g1[:, None])
    wv = np.ascontiguousarray(wv * g1[:, None])
    w1f = np.asarray(inputs["W1"], np.float32)
    b1 = np.asarray(inputs["b1"], np.float32).reshape(DFF) + b2n @ w1f
    w1 = (w1f * g2[:, None]).astype(ml_dtypes.bfloat16)
    wo = np.asarray(inputs["Wo"], np.float32).astype(ml_dtypes.bfloat16)
    w2 = np.asarray(inputs["W2"], np.float32).astype(ml_dtypes.bfloat16)
    shared = {
        "wq": wq, "wk": wk, "wv": wv, "wo": wo, "w1": w1, "w2": w2,
        "bq": bq, "bk": bk, "bv": bv,
        "bo": np.asarray(inputs["bo"], np.float32).reshape(C),
        "b1": b1,
        "b2": np.asarray(inputs["b2"], np.float32).reshape(C),
    }
    in_maps = []
    for c in range(N_CORES):
        b, half = c // 2, c % 2
        own = x[b, half * TQ:(half + 1) * TQ]
        other = x[b, (1 - half) * TQ:(2 - half) * TQ]
        x_perm = np.ascontiguousarray(np.concatenate([own, other], axis=0))
        in_maps.append(dict(shared, x=x_perm))
    return in_maps


def _run(inputs, **spmd_kwargs):
    nc = _get_nc()
    in_maps = _shard_inputs(inputs)
    res = run_bass_kernel_spmd(nc, in_maps, core_ids=list(range(N_CORES)),
                               **spmd_kwargs)
    out = np.empty((B, T, C), np.float32)
    for c in range(N_CORES):
        b, half = c // 2, c % 2
        out[b, half * TQ:(half + 1) * TQ] = res.results[c]["out"]
    return out, res


def kernel(**inputs) -> np.ndarray:
    out, _ = _run(inputs)
    return out


# revision 3
# speedup vs baseline: 1.0178x; 1.0150x over previous
"""Trainium2 Bass kernel for a pre-norm transformer block (MHSA + FFN), v2.

Sharding: 8 cores, data parallel over (batch, seq-half). Core c handles
batch c//2, sequence half c%2; K/V run over the full 2048 tokens
(softmax is permutation invariant, own tokens permuted first).

v2 vs v1:
- LN1 tiles interleave with pair-0 QKV so the PE starts early.
- Attention: ACT does ONLY exp, batched N=1024 (one activation per two
  score psum banks), probs written bf16; Q/K/V psum drains on DVE.
- PV keeps the fused ones-column denominator; drained via DVE
  reciprocal + DMA partition-broadcast + DVE multiply (no per-head
  transposes). Odd heads reach partitions 64-127 via SBUF->SBUF DMA.
- oT stays in SBUF (bf16); Wo/W1 bf16 paired with bf16 rhs.
- Stage D residual/LN2 interleaves with Wo ch1 / FFN-up; stage F
  re-streams W2 per 512-token chunk so chunk-0 epilogue overlaps
  chunk-1 matmuls.
- PSUM: attention = sc(4 banks) + qkv(2) + pv(2); D/E/F = ps(4)+pst(4).
"""
import contextlib

import numpy as np
import ml_dtypes

import concourse.bass as bass
import concourse.tile as tile
import concourse.mybir as mybir
from concourse.bass_utils import run_bass_kernel_spmd
from concourse.masks import make_identity

B, T, C = 4, 2048, 1024
H, DH = 16, 64
DFF = 4 * C
N_CORES = 8
TQ = T // 2
TS = T
NKO = C // 128
F32R = mybir.dt.float32r
F32 = mybir.dt.float32
BF16 = mybir.dt.bfloat16
EXP_BIAS = -128.0
EPS = 1e-5
AF = mybir.ActivationFunctionType
ALU = mybir.AluOpType

_ev_counter = [0]


def _legalize_sem_waits(nc):
    """This walrus build accepts at most 1 sem-wait per regular
    instruction (2 per InstEventSemaphore); split the rest ourselves."""
    for func in nc.m.functions:
        for bb in func.blocks:
            new = []
            changed = False
            for inst in bb.instructions:
                si = inst.sync_info
                cap = 2 if isinstance(inst, mybir.InstEventSemaphore) else 1
                if si is not None and len(si.on_wait) > cap:
                    waits = list(si.on_wait)
                    for i in range(cap, len(waits), 2):
                        _ev_counter[0] += 1
                        e = mybir.InstEventSemaphore(
                            name=f"EVSPLIT-{_ev_counter[0]}", ins=[], outs=[])
                        e.engine = inst.engine
                        e.sync_info = mybir.SyncInfo(
                            on_wait=waits[i:i + 2], on_update=[])
                        new.append(e)
                    inst.sync_info = mybir.SyncInfo(
                        on_wait=waits[:cap], on_update=list(si.on_update))
                    changed = True
                new.append(inst)
            if changed:
                bb.instructions = new


def _bcast_row(ap, p):
    """Broadcast a [1, n] AP across p partitions (DMA source)."""
    return bass.AP(tensor=ap.tensor, offset=ap.offset,
                   ap=[[0, p]] + [list(x) for x in ap.ap[1:]])


def _bcast_dram(ap, p=128):
    return bass.AP(tensor=ap.tensor, offset=ap.offset,
                   ap=[[0, p]] + [list(x) for x in ap.ap])


def _build_nc():
    nc = bass.Bass()

    x_d = nc.dram_tensor("x", [T, C], F32, kind="ExternalInput")
    wq_d = nc.dram_tensor("wq", [C, C], F32R, kind="ExternalInput")
    wk_d = nc.dram_tensor("wk", [C, C], F32R, kind="ExternalInput")
    wv_d = nc.dram_tensor("wv", [C, C], F32R, kind="ExternalInput")
    wo_d = nc.dram_tensor("wo", [C, C], BF16, kind="ExternalInput")
    w1_d = nc.dram_tensor("w1", [C, DFF], BF16, kind="ExternalInput")
    w2_d = nc.dram_tensor("w2", [DFF, C], BF16, kind="ExternalInput")
    bq_d = nc.dram_tensor("bq", [C], F32, kind="ExternalInput")
    bk_d = nc.dram_tensor("bk", [C], F32, kind="ExternalInput")
    bv_d = nc.dram_tensor("bv", [C], F32, kind="ExternalInput")
    bo_d = nc.dram_tensor("bo", [C], F32, kind="ExternalInput")
    b1_d = nc.dram_tensor("b1", [DFF], F32, kind="ExternalInput")
    b2_d = nc.dram_tensor("b2", [C], F32, kind="ExternalInput")
    out_d = nc.dram_tensor("out", [TQ, C], F32, kind="ExternalOutput")

    dr_h = nc.dram_tensor("dr_h", [H, 2, 512], F32)

    wq_r = wq_d.rearrange("(o p) f -> p o f", p=128)
    wk_r = wk_d.rearrange("(o p) f -> p o f", p=128)
    wv_r = wv_d.rearrange("(o p) f -> p o f", p=128)
    wo_r = wo_d.rearrange("(o p) f -> p o f", p=128)
    w1_r = w1_d.rearrange("(o p) f -> p o f", p=128)
    w2_r = w2_d.rearrange("(o p) f -> p o f", p=128)

    with tile.TileContext(nc) as tc:
        with contextlib.ExitStack() as top:
            consts = top.enter_context(tc.tile_pool(name="consts", bufs=1))
            stats = top.enter_context(tc.tile_pool(name="stats", bufs=8))

            identF = consts.tile([128, 128], F32, tag="identf")
            make_identity(nc, identF)
            identR = consts.tile([128, 128], F32R, tag="identr")
            nc.vector.tensor_copy(out=identR[:], in_=identF[:])
            ebias = consts.tile([128, 1], F32, tag="ebias")
            nc.vector.memset(ebias[:], EXP_BIAS)
            eps_t = consts.tile([128, 1], F32, tag="eps")
            nc.vector.memset(eps_t[:], EPS)
            bq_s = consts.tile([128, NKO], F32, tag="bq")
            bk_s = consts.tile([128, NKO], F32, tag="bk")
            bo_s = consts.tile([128, NKO], F32, tag="bo")
            b2_s = consts.tile([128, NKO], F32, tag="b2")
            b1_s = consts.tile([128, DFF // 128], F32, tag="b1")
            for dst, src in ((bq_s, bq_d), (bk_s, bk_d), (bo_s, bo_d),
                             (b2_s, b2_d), (b1_s, b1_d)):
                nc.sync.dma_start(out=dst[:],
                                  in_=src.rearrange("(o p) -> p o", p=128))
            bv_r = consts.tile([128, C], F32, tag="bvr")
            nc.gpsimd.dma_start(out=bv_r[:], in_=_bcast_dram(bv_d[:]))

            def layernorm(x_ap, out_ap):
                st = stats.tile([128, 2, 6], F32, tag="bnstats")
                mv = stats.tile([128, 2], F32, tag="bnaggr")
                xg = x_ap.rearrange("p (s d) -> p s d", s=2)
                for s in range(2):
                    nc.vector.bn_stats(out=st[:, s, :], in_=xg[:, s, :])
                nc.vector.bn_aggr(out=mv[:], in_=st[:])
                rstd = stats.tile([128, 1], F32, tag="rstd")
                nc.scalar.activation(out=rstd[:], in_=mv[:, 1:2],
                                     func=AF.Sqrt, bias=eps_t[:], scale=1.0)
                nc.vector.reciprocal(out=rstd[:], in_=rstd[:])
                nc.gpsimd.tensor_scalar(out=out_ap, in0=x_ap,
                                        scalar1=mv[:, 0:1], scalar2=rstd[:],
                                        op0=ALU.subtract, op1=ALU.mult)

            # oT outlives the attention scope (consumed by Wo)
            oTp = top.enter_context(tc.tile_pool(name="oTp", bufs=1))
            oT = oTp.tile([128, NKO, TQ], BF16, tag="oT")

            # =========== Attention phase ===========
            with contextlib.ExitStack() as abc:
                qkp = abc.enter_context(tc.tile_pool(name="qkp", bufs=2))
                vgp = abc.enter_context(tc.tile_pool(name="vgp", bufs=2))
                prb = abc.enter_context(tc.tile_pool(name="prb", bufs=2))
                drp = abc.enter_context(tc.tile_pool(name="drp", bufs=2))
                scp = abc.enter_context(
                    tc.tile_pool(name="scp", bufs=2, space="PSUM"))
                qkvp = abc.enter_context(
                    tc.tile_pool(name="qkvp", bufs=2, space="PSUM"))
                pvp = abc.enter_context(
                    tc.tile_pool(name="pvp", bufs=2, space="PSUM"))

                qk_tiles = {}
                vg_tiles = {}

                qkvscope = contextlib.ExitStack()
                if True:
                    xnp = qkvscope.enter_context(
                        tc.tile_pool(name="xnp", bufs=1))
                    xnT = xnp.tile([128, NKO, T], F32R, tag="xnT")
                    workA = qkvscope.enter_context(
                        tc.tile_pool(name="workA", bufs=3))
                    wgp = qkvscope.enter_context(
                        tc.tile_pool(name="wgp", bufs=2))

                    def a_unit(t):
                        x_t = workA.tile([128, C], F32, tag="x_t")
                        if t < 2:
                            with tc.high_priority():
                                nc.scalar.dma_start(
                                    out=x_t[:],
                                    in_=x_d[t * 128:(t + 1) * 128, :])
                        else:
                            nc.scalar.dma_start(
                                out=x_t[:],
                                in_=x_d[t * 128:(t + 1) * 128, :])
                        xn_r = workA.tile([128, C], F32R, tag="xn_r")
                        layernorm(x_t[:], xn_r[:])
                        pt = scp.tile([128, NKO, 128], F32R, tag="sc",
                                      name="pt")
                        for c in range(NKO):
                            nc.tensor.transpose(
                                pt[:, c, :],
                                xn_r[:, c * 128:(c + 1) * 128], identR[:])
                        nc.scalar.activation(
                            out=xnT[:, :, t * 128:(t + 1) * 128],
                            in_=pt[:], func=AF.Identity,
                            bias=0.0, scale=1.0)

                    def qk_gen(p):
                        wqt = wgp.tile([128, NKO, 128], F32R, tag="wqt",
                                       bufs=1)
                        wkt = wgp.tile([128, NKO, 128], F32R, tag="wkt",
                                       bufs=1)
                        nc.sync.dma_start(
                            out=wqt[:], in_=wq_r[:, :, p * 128:(p + 1) * 128])
                        nc.sync.dma_start(
                            out=wkt[:], in_=wk_r[:, :, p * 128:(p + 1) * 128])
                        qp = qkp.tile([128, TQ], F32R, tag="qp")
                        kp = qkp.tile([128, TS], F32R, tag="kp")
                        qk_tiles[p] = (qp, kp)
                        yield

                        def qu(ch):
                            pq = qkvp.tile([128, 512], F32, tag="qkv",
                                           name="pq")
                            for ko in range(NKO):
                                nc.tensor.matmul(
                                    pq[:], wqt[:, ko, :],
                                    xnT[:, ko, ch * 512:(ch + 1) * 512],
                                    start=(ko == 0), stop=(ko == NKO - 1))
                            nc.vector.tensor_scalar_add(
                                out=qp[:, ch * 512:(ch + 1) * 512],
                                in0=pq[:], scalar1=bq_s[:, p:p + 1])

                        def ku(ch):
                            pk = qkvp.tile([128, 512], F32, tag="qkv",
                                           name="pk")
                            for ko in range(NKO):
                                nc.tensor.matmul(
                                    pk[:], wkt[:, ko, :],
                                    xnT[:, ko, ch * 512:(ch + 1) * 512],
                                    start=(ko == 0), stop=(ko == NKO - 1))
                            nc.vector.tensor_scalar_add(
                                out=kp[:, ch * 512:(ch + 1) * 512],
                                in0=pk[:], scalar1=bk_s[:, p:p + 1])

                        for kind, ch in (("q", 0), ("k", 0), ("q", 1),
                                         ("k", 1), ("k", 2), ("k", 3)):
                            qu(ch) if kind == "q" else ku(ch)
                            yield

                    def v_gen(g):
                        wvt = wgp.tile([128, NKO, 256], F32R, tag="wvt",
                                       bufs=1)
                        nc.sync.dma_start(
                            out=wvt[:], in_=wv_r[:, :, g * 256:(g + 1) * 256])
                        vg = vgp.tile([128, TS // 128, 4, 65], BF16, tag="vg")
                        vg_tiles[g] = vg
                        nc.vector.memset(vg[:, :, :, 64:65], 1.0)
                        yield
                        for j in range(TS // 256):
                            pvv = qkvp.tile([128, 512], F32, tag="qkv")
                            for i in range(2):
                                to = 2 * j + i
                                for ko in range(NKO):
                                    nc.tensor.matmul(
                                        pvv[0:128, i * 256:(i + 1) * 256],
                                        xnT[:, ko, to * 128:(to + 1) * 128],
                                        wvt[:, ko, :],
                                        start=(ko == 0), stop=(ko == NKO - 1))
                            for i in range(2):
                                to = 2 * j + i
                                nc.vector.tensor_tensor(
                                    out=vg[:, to, :, 0:64],
                                    in0=pvv[:, i * 256:(i + 1) * 256].rearrange(
                                        "p (h d) -> p h d", d=64),
                                    in1=bv_r[:, g * 256:(g + 1) * 256].rearrange(
                                        "p (h d) -> p h d", d=64),
                                    op=ALU.add)
                            yield

                    # ---- prologue: LN1 tiles interleaved with pair-0 QKV
                    q0 = qk_gen(0)
                    next(q0)                    # wq/wk DMAs in flight
                    v0 = v_gen(0)
                    next(v0)                    # wv DMA + ones memset
                    for t in range(4):
                        a_unit(t)
                    next(q0)                    # Q(0,0)
                    next(q0)                    # K(0,0)
                    for t in range(4, 8):
                        a_unit(t)
                    next(q0)                    # Q(0,1)
                    next(q0)                    # K(0,1)
                    next(v0)                    # V(0,0)
                    next(v0)                    # V(0,1)
                    for t in range(8, 12):
                        a_unit(t)
                    next(q0)                    # K(0,2)
                    next(v0)                    # V(0,2)
                    next(v0)                    # V(0,3)
                    for t in range(12, 16):
                        a_unit(t)
                    next(q0)                    # K(0,3)
                    for _ in v0:
                        pass
                    for _ in q0:
                        pass

                    # ---- pair loop with QKV pulled between attention units
                    pull_src = {
                        0: [qk_gen(1)],
                        1: [qk_gen(2), v_gen(1)],
                        2: [qk_gen(3)],
                        3: [qk_gen(4), v_gen(2)],
                        4: [qk_gen(5)],
                        5: [qk_gen(6), v_gen(3)],
                        6: [qk_gen(7)],
                        7: [],
                    }

                    def attention_hc(h, ch, pull, pending):
                        p = h // 2
                        base = (h % 2) * 64
                        hl = h % 4
                        qp, kp = qk_tiles[p]
                        vg = vg_tiles[h // 4]
                        probsT = prb.tile([128, TS // 128, 512], BF16,
                                          tag="probs")
                        pvt = pvp.tile([128, 512], F32, tag="pv")

                        def s_unit(g):
                            sct = scp.tile([128, 1024], F32, tag="sc")
                            for i in range(2):
                                tso = 2 * g + i
                                nc.tensor.matmul(
                                    sct[:, i * 512:(i + 1) * 512],
                                    kp[base:base + DH,
                                       tso * 128:(tso + 1) * 128],
                                    qp[base:base + DH,
                                       ch * 512:(ch + 1) * 512],
                                    start=True, stop=True)
                            nc.scalar.activation(
                                out=probsT[:, 2 * g:2 * g + 2, :], in_=sct[:],
                                func=AF.Exp, scale=8.0, bias=ebias[:])

                        def p_unit(g):
                            for i in range(2):
                                to = 2 * g + i
                                nc.tensor.matmul(
                                    pvt[0:DH + 1, :],
                                    vg[:, to, hl, 0:DH + 1],
                                    probsT[:, to, :],
                                    start=(to == 0), stop=(to == 15))

                        s_unit(0)
                        s_unit(1)
                        if pending:
                            pending[0]()
                            pending.clear()
                        for g in range(6):
                            pull()
                            p_unit(g)
                            s_unit(g + 2)
                        p_unit(6)
                        p_unit(7)
                        pull()

                        def drain():
                        # denominator: reciprocal of psum row 64 (stays on
                        # partition 64), then DMA-broadcast to partitions
                        # 0-63 (compute engines cannot shift partitions)
                            rc = drp.tile([128, 512], F32, tag="rbc",
                                          name="rc")
                            nc.vector.reciprocal(out=rc[64:65, :],
                                                 in_=pvt[64:65, :])
                            nc.gpsimd.dma_start(out=dr_h[h, ch, :],
                                                in_=rc[64:65, :])
                            nc.gpsimd.dma_start(
                                out=rc[0:64, :],
                                in_=_bcast_dram(dr_h[h, ch, :], 64))
                            rb = rc
                            if h % 2 == 0:
                                nc.vector.tensor_tensor(
                                    out=oT[0:64, p,
                                           ch * 512:(ch + 1) * 512],
                                    in0=pvt[0:64, :], in1=rb[0:64, :],
                                    op=ALU.mult)
                            else:
                                so = drp.tile([64, 512], BF16, tag="so",
                                              name="so")
                                nc.vector.tensor_tensor(
                                    out=so[0:64, :], in0=pvt[0:64, :],
                                    in1=rb[0:64, :], op=ALU.mult)
                                nc.gpsimd.dma_start(
                                    out=oT[64:128, p,
                                           ch * 512:(ch + 1) * 512],
                                    in_=so[0:64, :])
                        pending.append(drain)

                    pending_drain = []
                    wo_tiles = {}
                    w1_tiles = {}
                    x_tiles = {}
                    x2_tiles = {}
                    for p in range(8):
                        gens = pull_src[p]

                        def pull():
                            while gens:
                                try:
                                    next(gens[0])
                                    return
                                except StopIteration:
                                    gens.pop(0)

                        for h in (2 * p, 2 * p + 1):
                            for ch in range(2):
                                attention_hc(h, ch, pull, pending_drain)
                        # generators must drain within their pair so the
                        # xnT/wgp scope can close after pair 6
                        if p >= 5:
                            pull()
                            while gens:
                                pull()
                        if p == 6:
                            # xnT/workA/wgp are dead: free them and pull
                            # the stage-D working set in under pair 7
                            qkvscope.close()
                            wop = top.enter_context(
                                tc.tile_pool(name="wop", bufs=2,
                                             side="right"))
                            w1p = top.enter_context(
                                tc.tile_pool(name="w1p", bufs=6,
                                             side="right"))
                            x2w = top.enter_context(
                                tc.tile_pool(name="x2w", bufs=4,
                                             side="right"))

                            def load_wo(k):
                                if k >= 16 or k in wo_tiles:
                                    return
                                f = k % NKO
                                wot = wop.tile([128, NKO, 128], BF16,
                                               tag="wo", name="wot")
                                nc.sync.dma_start(
                                    out=wot[:],
                                    in_=wo_r[:, :, f * 128:(f + 1) * 128])
                                wo_tiles[k] = wot

                            def load_w1(fb):
                                if fb >= DFF // 256 or fb in w1_tiles:
                                    return
                                w1t = w1p.tile([128, NKO, 256], BF16,
                                               tag="w1t", name="w1t")
                                nc.sync.dma_start(
                                    out=w1t[:],
                                    in_=w1_r[:, :, fb * 256:(fb + 1) * 256])
                                w1_tiles[fb] = w1t

                            def load_x(t):
                                x_t = x2w.tile([128, C], F32, tag="x_t",
                                               name="x_t")
                                nc.scalar.dma_start(
                                    out=x_t[:],
                                    in_=x_d[t * 128:(t + 1) * 128, :])
                                x_tiles[t] = x_t
                                x2_tiles[t] = x2w.tile(
                                    [128, C], F32, tag="x2_t",
                                    name="x2_t", bufs=8)

                            for k in range(4):
                                load_wo(k)
                            load_w1(0)
                            load_w1(1)
                            for t in range(8):
                                load_x(t)
                    if pending_drain:
                        pending_drain[0]()
                        pending_drain.clear()

            # =========== Stages D/E/F ===========
            with contextlib.ExitStack() as dstk:
                w2p = dstk.enter_context(tc.tile_pool(name="w2p", bufs=2))
                fm = dstk.enter_context(tc.tile_pool(name="fm", bufs=1))
                workD = dstk.enter_context(tc.tile_pool(name="workD", bufs=2))
                psD = dstk.enter_context(
                    tc.tile_pool(name="psD", bufs=4, space="PSUM"))
                pstD = dstk.enter_context(
                    tc.tile_pool(name="pstD", bufs=4, space="PSUM"))

                w2_tiles = {}

                def load_w2(f):
                    if f >= NKO or f in w2_tiles:
                        return
                    w2t = w2p.tile([128, DFF // 128, 128], BF16, tag="w2t")
                    nc.sync.dma_start(
                        out=w2t[:], in_=w2_r[:, :, f * 128:(f + 1) * 128])
                    w2_tiles[f] = w2t

                xn2T = fm.tile([128, NKO, TQ], BF16, tag="xn2T")
                h1T = fm.tile([128, DFF // 128, TQ], BF16, tag="h1T")

                def wo_unit(f, ch):
                    k = ch * NKO + f
                    load_wo(k + 2)
                    ps = psD.tile([128, 512], F32, tag="ps")
                    for ko in range(NKO):
                        nc.tensor.matmul(
                            ps[:], wo_tiles[k][:, ko, :],
                            oT[:, ko, ch * 512:(ch + 1) * 512],
                            start=(ko == 0), stop=(ko == NKO - 1))
                    s = workD.tile([128, 512], F32R, tag="sfc", bufs=3)
                    nc.scalar.activation(out=s[:], in_=ps[:],
                                         func=AF.Identity,
                                         bias=bo_s[:, f:f + 1], scale=1.0)
                    pt = pstD.tile([128, 4, 128], F32R, tag="pst")
                    for tt in range(4):
                        t = ch * 4 + tt
                        nc.tensor.transpose(pt[:, tt, :],
                                            s[:, tt * 128:(tt + 1) * 128],
                                            identR[:])
                        nc.vector.tensor_tensor(
                            out=x2_tiles[t][:, f * 128:(f + 1) * 128],
                            in0=pt[:, tt, :],
                            in1=x_tiles[t][:, f * 128:(f + 1) * 128],
                            op=ALU.add)

                def d_finish(t):
                    x2_t = x2_tiles[t]
                    x_tiles.pop(t)
                    xn2_r = workD.tile([128, C], F32R, tag="xn2_r",
                                       bufs=1)
                    layernorm(x2_t[:], xn2_r[:])
                    for half in range(2):
                        pt = pstD.tile([128, 4, 128], F32R, tag="pst")
                        for j in range(4):
                            c = half * 4 + j
                            nc.tensor.transpose(
                                pt[:, j, :], xn2_r[:, c * 128:(c + 1) * 128],
                                identR[:])
                        nc.scalar.activation(
                            out=xn2T[:, half * 4:(half + 1) * 4,
                                     t * 128:(t + 1) * 128],
                            in_=pt[:], func=AF.Identity,
                            bias=0.0, scale=1.0)

                def e_unit(fb, fs, ch):
                    f = fb * 2 + fs
                    ps = psD.tile([128, 512], F32, tag="ps")
                    for ko in range(NKO):
                        nc.tensor.matmul(
                            ps[:], w1_tiles[fb][:, ko, fs * 128:(fs + 1) * 128],
                            xn2T[:, ko, ch * 512:(ch + 1) * 512],
                            start=(ko == 0), stop=(ko == NKO - 1))
                    nc.scalar.activation(out=h1T[:, f, ch * 512:(ch + 1) * 512],
                                         in_=ps[:], func=AF.Relu,
                                         bias=b1_s[:, f:f + 1], scale=1.0)

                def f_unit(f, ch, finish=False):
                    load_w2(f + 2)
                    halves = (0, 1) if finish else (None,)
                    for hv in halves:
                        lo = 0 if hv in (None, 0) else 256
                        hi = 512 if hv in (None, 1) else 256
                        ps = psD.tile([128, 512], F32, tag="ps", name="ps")
                        for ko in range(DFF // 128):
                            nc.tensor.matmul(
                                ps[:, 0:hi - lo], w2_tiles[f][:, ko, :],
                                h1T[:, ko, ch * 512 + lo:ch * 512 + hi],
                                start=(ko == 0),
                                stop=(ko == DFF // 128 - 1))
                        s = workD.tile([128, 512], F32R, tag="sfc", bufs=3,
                                       name="s")
                        nc.scalar.activation(out=s[:, 0:hi - lo],
                                             in_=ps[:, 0:hi - lo],
                                             func=AF.Identity,
                                             bias=b2_s[:, f:f + 1], scale=1.0)
                        pt = pstD.tile([128, 4, 128], F32R, tag="pst",
                                       name="pt")
                        for tt in range((hi - lo) // 128):
                            t = ch * 4 + (lo // 128) + tt
                            nc.tensor.transpose(
                                pt[:, tt, :],
                                s[:, tt * 128:(tt + 1) * 128], identR[:])
                            nc.vector.tensor_tensor(
                                out=x2_tiles[t][:, f * 128:(f + 1) * 128],
                                in0=pt[:, tt, :],
                                in1=x2_tiles[t][:, f * 128:(f + 1) * 128],
                                op=ALU.add)
                            if finish:
                                f_finish(t)

                def f_finish(t):
                    out_t = x2_tiles.pop(t)
                    eng = nc.gpsimd if t % 2 == 0 else nc.sync
                    eng.dma_start(out=out_d[t * 128:(t + 1) * 128, :],
                                  in_=out_t[:])

                # ---- Stage D: Wo + residual + LN2 ----
                for f in range(NKO):
                    wo_unit(f, 0)
                # ch1 Wo interleaved with ch0 LN2 finish; d_finish(t) must
                # precede wo_unit(t, 1) so the x2w buffer-rotation WAR sems
                # point backwards in each engine's stream (no deadlock)
                for f in range(NKO):
                    if f < 4:
                        d_finish(f)
                    wo_unit(f, 1)
                # ---- Stage E: phase 1 = fb0-3 ch0 only, interleaved with
                # the ch1 LN2 finishes; phase 2 = fb0-3 ch1; phase 3 = rest.
                # (w1p bufs=6 keeps fb0-3 alive until phase 2.) ----
                for fb in range(4):
                    load_w1(fb + 2)
                    e_unit(fb, 0, 0)
                    e_unit(fb, 1, 0)
                    d_finish(4 + fb)
                for fb in range(4):
                    load_w1(6 + fb)
                    e_unit(fb, 0, 1)
                    e_unit(fb, 1, 1)
                for fb in range(4, DFF // 256):
                    load_w1(fb + 2)
                    for fs in range(2):
                        for ch in range(2):
                            e_unit(fb, fs, ch)
                load_w2(0)
                load_w2(1)
                # ---- Stage F: f-outer, W2 loaded once, residual in place
                for f in range(NKO):
                    f_unit(f, 0, finish=(f == NKO - 1))
                    f_unit(f, 1, finish=(f == NKO - 1))

    nc.finalize()
    _legalize_sem_waits(nc)
    return nc


_NC_CACHE = None


def _get_nc():
    global _NC_CACHE
    if _NC_CACHE is None:
        _NC_CACHE = _build_nc()
    return _NC_CACHE


def _shard_inputs(inputs):
    x = np.asarray(inputs["x"], np.float32)
    # Fold the LN affine (g, b) into the following linear layers:
    #   (z*g + b) @ W + c  ==  z @ (diag(g) W) + (c + b @ W)
    g1 = np.asarray(inputs["ln1_g"], np.float32).reshape(C)
    b1n = np.asarray(inputs["ln1_b"], np.float32).reshape(C)
    g2 = np.asarray(inputs["ln2_g"], np.float32).reshape(C)
    b2n = np.asarray(inputs["ln2_b"], np.float32).reshape(C)
    wq = np.ascontiguousarray(
        np.transpose(np.asarray(inputs["Wq"], np.float32), (1, 0, 2)).reshape(C, C))
    wk = np.ascontiguousarray(
        np.transpose(np.asarray(inputs["Wk"], np.float32), (1, 0, 2)).reshape(C, C))
    wv = np.ascontiguousarray(
        np.transpose(np.asarray(inputs["Wv"], np.float32), (1, 0, 2)).reshape(C, C))
    bq = np.asarray(inputs["bq"], np.float32).reshape(C) + b1n @ wq
    bk = np.asarray(inputs["bk"], np.float32).reshape(C) + b1n @ wk
    bv = np.asarray(inputs["bv"], np.float32).reshape(C) + b1n @ wv
    wq = np.ascontiguousarray(wq * g1[:, None])
    wk = np.ascontiguousarray(wk * g1[:, None])
    wv = np.ascontiguousarray(wv * g1[:, None])
    w1f = np.asarray(inputs["W1"], np.float32)
    b1 = np.asarray(inputs["b1"], np.float32).reshape(DFF) + b2n @ w1f
    w1 = (w1f * g2[:, None]).astype(ml_dtypes.bfloat16)
    wo = np.asarray(inputs["Wo"], np.float32).astype(ml_dtypes.bfloat16)
    w2 = np.asarray(inputs["W2"], np.float32).astype(ml_dtypes.bfloat16)
    shared = {
        "wq": wq, "wk": wk, "wv": wv, "wo": wo, "w1": w1, "w2": w2,
        "bq": bq, "bk": bk, "bv": bv,
        "bo": np.asarray(inputs["bo"], np.float32).reshape(C),
        "b1": b1,
        "b2": np.asarray(inputs["b2"], np.float32).reshape(C),
    }
    in_maps = []
    for c in range(N_CORES):
        b, half = c // 2, c % 2
        own = x[b, half * TQ:(half + 1) * TQ]
        other = x[b, (1 - half) * TQ:(2 - half) * TQ]
        x_perm = np.ascontiguousarray(np.concatenate([own, other], axis=0))
        in_maps.append(dict(shared, x=x_perm))
    return in_maps


def _run(inputs, **spmd_kwargs):
    nc = _get_nc()
    in_maps = _shard_inputs(inputs)
    res = run_bass_kernel_spmd(nc, in_maps, core_ids=list(range(N_CORES)),
                               **spmd_kwargs)
    out = np.empty((B, T, C), np.float32)
    for c in range(N_CORES):
        b, half = c // 2, c % 2
        out[b, half * TQ:(half + 1) * TQ] = res.results[c]["out"]
    return out, res


def kernel(**inputs) -> np.ndarray:
    out, _ = _run(inputs)
    return out


# revision 4
# speedup vs baseline: 1.0218x; 1.0039x over previous
"""Trainium2 Bass kernel for a pre-norm transformer block (MHSA + FFN), v2.

Sharding: 8 cores, data parallel over (batch, seq-half). Core c handles
batch c//2, sequence half c%2; K/V run over the full 2048 tokens
(softmax is permutation invariant, own tokens permuted first).

v2 vs v1:
- LN1 tiles interleave with pair-0 QKV so the PE starts early.
- Attention: ACT does ONLY exp, batched N=1024 (one activation per two
  score psum banks), probs written bf16; Q/K/V psum drains on DVE.
- PV keeps the fused ones-column denominator; drained via DVE
  reciprocal + DMA partition-broadcast + DVE multiply (no per-head
  transposes). Odd heads reach partitions 64-127 via SBUF->SBUF DMA.
- oT stays in SBUF (bf16); Wo/W1 bf16 paired with bf16 rhs.
- Stage D residual/LN2 interleaves with Wo ch1 / FFN-up; stage F
  re-streams W2 per 512-token chunk so chunk-0 epilogue overlaps
  chunk-1 matmuls.
- PSUM: attention = sc(4 banks) + qkv(2) + pv(2); D/E/F = ps(4)+pst(4).
"""
import contextlib

import numpy as np
import ml_dtypes

import concourse.bass as bass
import concourse.tile as tile
import concourse.mybir as mybir
from concourse.bass_utils import run_bass_kernel_spmd
from concourse.masks import make_identity

B, T, C = 4, 2048, 1024
H, DH = 16, 64
DFF = 4 * C
N_CORES = 8
TQ = T // 2
TS = T
NKO = C // 128
F32R = mybir.dt.float32r
F32 = mybir.dt.float32
BF16 = mybir.dt.bfloat16
EXP_BIAS = -128.0
EPS = 1e-5
AF = mybir.ActivationFunctionType
ALU = mybir.AluOpType

_ev_counter = [0]


def _legalize_sem_waits(nc):
    """This walrus build accepts at most 1 sem-wait per regular
    instruction (2 per InstEventSemaphore); split the rest ourselves."""
    for func in nc.m.functions:
        for bb in func.blocks:
            new = []
            changed = False
            for inst in bb.instructions:
                si = inst.sync_info
                cap = 2 if isinstance(inst, mybir.InstEventSemaphore) else 1
                if si is not None and len(si.on_wait) > cap:
                    waits = list(si.on_wait)
                    for i in range(cap, len(waits), 2):
                        _ev_counter[0] += 1
                        e = mybir.InstEventSemaphore(
                            name=f"EVSPLIT-{_ev_counter[0]}", ins=[], outs=[])
                        e.engine = inst.engine
                        e.sync_info = mybir.SyncInfo(
                            on_wait=waits[i:i + 2], on_update=[])
                        new.append(e)
                    inst.sync_info = mybir.SyncInfo(
                        on_wait=waits[:cap], on_update=list(si.on_update))
                    changed = True
                new.append(inst)
            if changed:
                bb.instructions = new


def _bcast_row(ap, p):
    """Broadcast a [1, n] AP across p partitions (DMA source)."""
    return bass.AP(tensor=ap.tensor, offset=ap.offset,
                   ap=[[0, p]] + [list(x) for x in ap.ap[1:]])


def _bcast_dram(ap, p=128):
    return bass.AP(tensor=ap.tensor, offset=ap.offset,
                   ap=[[0, p]] + [list(x) for x in ap.ap])


def _build_nc():
    nc = bass.Bass()

    x_d = nc.dram_tensor("x", [T, C], F32, kind="ExternalInput")
    wq_d = nc.dram_tensor("wq", [C, C], F32R, kind="ExternalInput")
    wk_d = nc.dram_tensor("wk", [C, C], F32R, kind="ExternalInput")
    wv_d = nc.dram_tensor("wv", [C, C], F32R, kind="ExternalInput")
    wo_d = nc.dram_tensor("wo", [C, C], BF16, kind="ExternalInput")
    w1_d = nc.dram_tensor("w1", [C, DFF], BF16, kind="ExternalInput")
    w2_d = nc.dram_tensor("w2", [DFF, C], BF16, kind="ExternalInput")
    bq_d = nc.dram_tensor("bq", [C], F32, kind="ExternalInput")
    bk_d = nc.dram_tensor("bk", [C], F32, kind="ExternalInput")
    bv_d = nc.dram_tensor("bv", [C], F32, kind="ExternalInput")
    bo_d = nc.dram_tensor("bo", [C], F32, kind="ExternalInput")
    b1_d = nc.dram_tensor("b1", [DFF], F32, kind="ExternalInput")
    b2_d = nc.dram_tensor("b2", [C], F32, kind="ExternalInput")
    out_d = nc.dram_tensor("out", [TQ, C], F32, kind="ExternalOutput")

    dr_h = nc.dram_tensor("dr_h", [H, 2, 512], F32)

    wq_r = wq_d.rearrange("(o p) f -> p o f", p=128)
    wk_r = wk_d.rearrange("(o p) f -> p o f", p=128)
    wv_r = wv_d.rearrange("(o p) f -> p o f", p=128)
    wo_r = wo_d.rearrange("(o p) f -> p o f", p=128)
    w1_r = w1_d.rearrange("(o p) f -> p o f", p=128)
    w2_r = w2_d.rearrange("(o p) f -> p o f", p=128)

    with tile.TileContext(nc) as tc:
        with contextlib.ExitStack() as top:
            consts = top.enter_context(tc.tile_pool(name="consts", bufs=1))
            stats = top.enter_context(tc.tile_pool(name="stats", bufs=8))

            identF = consts.tile([128, 128], F32, tag="identf")
            make_identity(nc, identF)
            identR = consts.tile([128, 128], F32R, tag="identr")
            nc.vector.tensor_copy(out=identR[:], in_=identF[:])
            ebias = consts.tile([128, 1], F32, tag="ebias")
            nc.vector.memset(ebias[:], EXP_BIAS)
            eps_t = consts.tile([128, 1], F32, tag="eps")
            nc.vector.memset(eps_t[:], EPS)
            bq_s = consts.tile([128, NKO], F32, tag="bq")
            bk_s = consts.tile([128, NKO], F32, tag="bk")
            bo_s = consts.tile([128, NKO], F32, tag="bo")
            b2_s = consts.tile([128, NKO], F32, tag="b2")
            b1_s = consts.tile([128, DFF // 128], F32, tag="b1")
            for dst, src in ((bq_s, bq_d), (bk_s, bk_d), (bo_s, bo_d),
                             (b2_s, b2_d), (b1_s, b1_d)):
                nc.sync.dma_start(out=dst[:],
                                  in_=src.rearrange("(o p) -> p o", p=128))
            bv_r = consts.tile([128, C], F32, tag="bvr")
            nc.gpsimd.dma_start(out=bv_r[:], in_=_bcast_dram(bv_d[:]))

            def layernorm(x_ap, out_ap):
                st = stats.tile([128, 2, 6], F32, tag="bnstats")
                mv = stats.tile([128, 2], F32, tag="bnaggr")
                xg = x_ap.rearrange("p (s d) -> p s d", s=2)
                for s in range(2):
                    nc.vector.bn_stats(out=st[:, s, :], in_=xg[:, s, :])
                nc.vector.bn_aggr(out=mv[:], in_=st[:])
                rstd = stats.tile([128, 1], F32, tag="rstd")
                nc.scalar.activation(out=rstd[:], in_=mv[:, 1:2],
                                     func=AF.Sqrt, bias=eps_t[:], scale=1.0)
                nc.vector.reciprocal(out=rstd[:], in_=rstd[:])
                nc.vector.tensor_scalar(out=out_ap, in0=x_ap,
                                        scalar1=mv[:, 0:1], scalar2=rstd[:],
                                        op0=ALU.subtract, op1=ALU.mult)

            # oT outlives the attention scope (consumed by Wo)
            oTp = top.enter_context(tc.tile_pool(name="oTp", bufs=1))
            oT = oTp.tile([128, NKO, TQ], BF16, tag="oT")

            # =========== Attention phase ===========
            with contextlib.ExitStack() as abc:
                qkp = abc.enter_context(tc.tile_pool(name="qkp", bufs=2))
                vgp = abc.enter_context(tc.tile_pool(name="vgp", bufs=2))
                prb = abc.enter_context(tc.tile_pool(name="prb", bufs=2))
                drp = abc.enter_context(tc.tile_pool(name="drp", bufs=2))
                scp = abc.enter_context(
                    tc.tile_pool(name="scp", bufs=2, space="PSUM"))
                qkvp = abc.enter_context(
                    tc.tile_pool(name="qkvp", bufs=2, space="PSUM"))
                pvp = abc.enter_context(
                    tc.tile_pool(name="pvp", bufs=2, space="PSUM"))

                qk_tiles = {}
                vg_tiles = {}

                qkvscope = contextlib.ExitStack()
                if True:
                    xnp = qkvscope.enter_context(
                        tc.tile_pool(name="xnp", bufs=1))
                    xnT = xnp.tile([128, NKO, T], F32R, tag="xnT")
                    workA = qkvscope.enter_context(
                        tc.tile_pool(name="workA", bufs=3))
                    wgp = qkvscope.enter_context(
                        tc.tile_pool(name="wgp", bufs=2))

                    def a_unit(t):
                        x_t = workA.tile([128, C], F32, tag="x_t")
                        if t < 2:
                            with tc.high_priority():
                                nc.scalar.dma_start(
                                    out=x_t[:],
                                    in_=x_d[t * 128:(t + 1) * 128, :])
                        else:
                            nc.scalar.dma_start(
                                out=x_t[:],
                                in_=x_d[t * 128:(t + 1) * 128, :])
                        xn_r = workA.tile([128, C], F32R, tag="xn_r")
                        layernorm(x_t[:], xn_r[:])
                        pt = scp.tile([128, NKO, 128], F32R, tag="sc",
                                      name="pt")
                        for c in range(NKO):
                            nc.tensor.transpose(
                                pt[:, c, :],
                                xn_r[:, c * 128:(c + 1) * 128], identR[:])
                        nc.scalar.activation(
                            out=xnT[:, :, t * 128:(t + 1) * 128],
                            in_=pt[:], func=AF.Identity,
                            bias=0.0, scale=1.0)

                    def qk_gen(p):
                        wqt = wgp.tile([128, NKO, 128], F32R, tag="wqt",
                                       bufs=1)
                        wkt = wgp.tile([128, NKO, 128], F32R, tag="wkt",
                                       bufs=1)
                        nc.sync.dma_start(
                            out=wqt[:], in_=wq_r[:, :, p * 128:(p + 1) * 128])
                        nc.sync.dma_start(
                            out=wkt[:], in_=wk_r[:, :, p * 128:(p + 1) * 128])
                        qp = qkp.tile([128, TQ], F32R, tag="qp")
                        kp = qkp.tile([128, TS], F32R, tag="kp")
                        qk_tiles[p] = (qp, kp)
                        yield

                        def qu(ch):
                            pq = qkvp.tile([128, 512], F32, tag="qkv",
                                           name="pq")
                            for ko in range(NKO):
                                nc.tensor.matmul(
                                    pq[:], wqt[:, ko, :],
                                    xnT[:, ko, ch * 512:(ch + 1) * 512],
                                    start=(ko == 0), stop=(ko == NKO - 1))
                            nc.vector.tensor_scalar_add(
                                out=qp[:, ch * 512:(ch + 1) * 512],
                                in0=pq[:], scalar1=bq_s[:, p:p + 1])

                        def ku(ch):
                            pk = qkvp.tile([128, 512], F32, tag="qkv",
                                           name="pk")
                            for ko in range(NKO):
                                nc.tensor.matmul(
                                    pk[:], wkt[:, ko, :],
                                    xnT[:, ko, ch * 512:(ch + 1) * 512],
                                    start=(ko == 0), stop=(ko == NKO - 1))
                            nc.vector.tensor_scalar_add(
                                out=kp[:, ch * 512:(ch + 1) * 512],
                                in0=pk[:], scalar1=bk_s[:, p:p + 1])

                        for kind, ch in (("q", 0), ("k", 0), ("q", 1),
                                         ("k", 1), ("k", 2), ("k", 3)):
                            qu(ch) if kind == "q" else ku(ch)
                            yield

                    def v_gen(g):
                        wvt = wgp.tile([128, NKO, 256], F32R, tag="wvt",
                                       bufs=1)
                        nc.sync.dma_start(
                            out=wvt[:], in_=wv_r[:, :, g * 256:(g + 1) * 256])
                        vg = vgp.tile([128, TS // 128, 4, 65], BF16, tag="vg")
                        vg_tiles[g] = vg
                        nc.vector.memset(vg[:, :, :, 64:65], 1.0)
                        yield
                        for j in range(TS // 256):
                            pvv = qkvp.tile([128, 512], F32, tag="qkv")
                            for i in range(2):
                                to = 2 * j + i
                                for ko in range(NKO):
                                    nc.tensor.matmul(
                                        pvv[0:128, i * 256:(i + 1) * 256],
                                        xnT[:, ko, to * 128:(to + 1) * 128],
                                        wvt[:, ko, :],
                                        start=(ko == 0), stop=(ko == NKO - 1))
                            for i in range(2):
                                to = 2 * j + i
                                nc.vector.tensor_tensor(
                                    out=vg[:, to, :, 0:64],
                                    in0=pvv[:, i * 256:(i + 1) * 256].rearrange(
                                        "p (h d) -> p h d", d=64),
                                    in1=bv_r[:, g * 256:(g + 1) * 256].rearrange(
                                        "p (h d) -> p h d", d=64),
                                    op=ALU.add)
                            yield

                    # ---- prologue: LN1 tiles interleaved with pair-0 QKV
                    q0 = qk_gen(0)
                    next(q0)                    # wq/wk DMAs in flight
                    v0 = v_gen(0)
                    next(v0)                    # wv DMA + ones memset
                    for t in range(4):
                        a_unit(t)
                    next(q0)                    # Q(0,0)
                    next(q0)                    # K(0,0)
                    for t in range(4, 8):
                        a_unit(t)
                    next(q0)                    # Q(0,1)
                    next(q0)                    # K(0,1)
                    next(v0)                    # V(0,0)
                    next(v0)                    # V(0,1)
                    for t in range(8, 12):
                        a_unit(t)
                    next(q0)                    # K(0,2)
                    next(v0)                    # V(0,2)
                    next(v0)                    # V(0,3)
                    for t in range(12, 16):
                        a_unit(t)
                    next(q0)                    # K(0,3)
                    for _ in v0:
                        pass
                    for _ in q0:
                        pass

                    # ---- pair loop with QKV pulled between attention units
                    pull_src = {
                        0: [qk_gen(1)],
                        1: [qk_gen(2), v_gen(1)],
                        2: [qk_gen(3)],
                        3: [qk_gen(4), v_gen(2)],
                        4: [qk_gen(5)],
                        5: [qk_gen(6), v_gen(3)],
                        6: [qk_gen(7)],
                        7: [],
                    }

                    def attention_hc(h, ch, pull, pending):
                        p = h // 2
                        base = (h % 2) * 64
                        hl = h % 4
                        qp, kp = qk_tiles[p]
                        vg = vg_tiles[h // 4]
                        probsT = prb.tile([128, TS // 128, 512], BF16,
                                          tag="probs")
                        pvt = pvp.tile([128, 512], F32, tag="pv")

                        def s_unit(g):
                            sct = scp.tile([128, 1024], F32, tag="sc")
                            for i in range(2):
                                tso = 2 * g + i
                                nc.tensor.matmul(
                                    sct[:, i * 512:(i + 1) * 512],
                                    kp[base:base + DH,
                                       tso * 128:(tso + 1) * 128],
                                    qp[base:base + DH,
                                       ch * 512:(ch + 1) * 512],
                                    start=True, stop=True)
                            nc.scalar.activation(
                                out=probsT[:, 2 * g:2 * g + 2, :], in_=sct[:],
                                func=AF.Exp, scale=8.0, bias=ebias[:])

                        def p_unit(g):
                            for i in range(2):
                                to = 2 * g + i
                                nc.tensor.matmul(
                                    pvt[0:DH + 1, :],
                                    vg[:, to, hl, 0:DH + 1],
                                    probsT[:, to, :],
                                    start=(to == 0), stop=(to == 15))

                        s_unit(0)
                        s_unit(1)
                        if pending:
                            pending[0]()
                            pending.clear()
                        for g in range(6):
                            pull()
                            p_unit(g)
                            s_unit(g + 2)
                        p_unit(6)
                        p_unit(7)
                        pull()

                        def drain():
                        # denominator: reciprocal of psum row 64 (stays on
                        # partition 64), then DMA-broadcast to partitions
                        # 0-63 (compute engines cannot shift partitions)
                            rc = drp.tile([128, 512], F32, tag="rbc",
                                          name="rc")
                            nc.vector.reciprocal(out=rc[64:65, :],
                                                 in_=pvt[64:65, :])
                            nc.gpsimd.dma_start(out=dr_h[h, ch, :],
                                                in_=rc[64:65, :])
                            nc.gpsimd.dma_start(
                                out=rc[0:64, :],
                                in_=_bcast_dram(dr_h[h, ch, :], 64))
                            rb = rc
                            if h % 2 == 0:
                                nc.vector.tensor_tensor(
                                    out=oT[0:64, p,
                                           ch * 512:(ch + 1) * 512],
                                    in0=pvt[0:64, :], in1=rb[0:64, :],
                                    op=ALU.mult)
                            else:
                                so = drp.tile([64, 512], BF16, tag="so",
                                              name="so")
                                nc.vector.tensor_tensor(
                                    out=so[0:64, :], in0=pvt[0:64, :],
                                    in1=rb[0:64, :], op=ALU.mult)
                                nc.gpsimd.dma_start(
                                    out=oT[64:128, p,
                                           ch * 512:(ch + 1) * 512],
                                    in_=so[0:64, :])
                        pending.append(drain)

                    pending_drain = []
                    wo_tiles = {}
                    w1_tiles = {}
                    x_tiles = {}
                    x2_tiles = {}
                    for p in range(8):
                        gens = pull_src[p]

                        def pull():
                            while gens:
                                try:
                                    next(gens[0])
                                    return
                                except StopIteration:
                                    gens.pop(0)

                        for h in (2 * p, 2 * p + 1):
                            for ch in range(2):
                                attention_hc(h, ch, pull, pending_drain)
                        # generators must drain within their pair so the
                        # xnT/wgp scope can close after pair 6
                        if p >= 5:
                            pull()
                            while gens:
                                pull()
                        if p == 6:
                            # xnT/workA/wgp are dead: free them and pull
                            # the stage-D working set in under pair 7
                            qkvscope.close()
                            wop = top.enter_context(
                                tc.tile_pool(name="wop", bufs=2,
                                             side="right"))
                            w1p = top.enter_context(
                                tc.tile_pool(name="w1p", bufs=6,
                                             side="right"))
                            x2w = top.enter_context(
                                tc.tile_pool(name="x2w", bufs=4,
                                             side="right"))

                            def load_wo(k):
                                if k >= 16 or k in wo_tiles:
                                    return
                                f = k % NKO
                                wot = wop.tile([128, NKO, 128], BF16,
                                               tag="wo", name="wot")
                                nc.sync.dma_start(
                                    out=wot[:],
                                    in_=wo_r[:, :, f * 128:(f + 1) * 128])
                                wo_tiles[k] = wot

                            def load_w1(fb):
                                if fb >= DFF // 256 or fb in w1_tiles:
                                    return
                                w1t = w1p.tile([128, NKO, 256], BF16,
                                               tag="w1t", name="w1t")
                                nc.sync.dma_start(
                                    out=w1t[:],
                                    in_=w1_r[:, :, fb * 256:(fb + 1) * 256])
                                w1_tiles[fb] = w1t

                            def load_x(t):
                                x_t = x2w.tile([128, C], F32, tag="x_t",
                                               name="x_t")
                                nc.scalar.dma_start(
                                    out=x_t[:],
                                    in_=x_d[t * 128:(t + 1) * 128, :])
                                x_tiles[t] = x_t
                                x2_tiles[t] = x2w.tile(
                                    [128, C], F32, tag="x2_t",
                                    name="x2_t", bufs=8)

                            for k in range(4):
                                load_wo(k)
                            load_w1(0)
                            load_w1(1)
                            for t in range(8):
                                load_x(t)
                    if pending_drain:
                        pending_drain[0]()
                        pending_drain.clear()

            # =========== Stages D/E/F ===========
            with contextlib.ExitStack() as dstk:
                w2p = dstk.enter_context(tc.tile_pool(name="w2p", bufs=2))
                fm = dstk.enter_context(tc.tile_pool(name="fm", bufs=1))
                workD = dstk.enter_context(tc.tile_pool(name="workD", bufs=2))
                psD = dstk.enter_context(
                    tc.tile_pool(name="psD", bufs=4, space="PSUM"))
                pstD = dstk.enter_context(
                    tc.tile_pool(name="pstD", bufs=4, space="PSUM"))

                w2_tiles = {}

                def load_w2(f):
                    if f >= NKO or f in w2_tiles:
                        return
                    w2t = w2p.tile([128, DFF // 128, 128], BF16, tag="w2t")
                    nc.sync.dma_start(
                        out=w2t[:], in_=w2_r[:, :, f * 128:(f + 1) * 128])
                    w2_tiles[f] = w2t

                xn2T = fm.tile([128, NKO, TQ], BF16, tag="xn2T")
                h1T = fm.tile([128, DFF // 128, TQ], BF16, tag="h1T")

                def wo_unit(f, ch):
                    k = ch * NKO + f
                    load_wo(k + 2)
                    ps = psD.tile([128, 512], F32, tag="ps")
                    for ko in range(NKO):
                        nc.tensor.matmul(
                            ps[:], wo_tiles[k][:, ko, :],
                            oT[:, ko, ch * 512:(ch + 1) * 512],
                            start=(ko == 0), stop=(ko == NKO - 1))
                    s = workD.tile([128, 512], F32R, tag="sfc", bufs=3)
                    nc.scalar.activation(out=s[:], in_=ps[:],
                                         func=AF.Identity,
                                         bias=bo_s[:, f:f + 1], scale=1.0)
                    pt = pstD.tile([128, 4, 128], F32R, tag="pst")
                    for tt in range(4):
                        t = ch * 4 + tt
                        nc.tensor.transpose(pt[:, tt, :],
                                            s[:, tt * 128:(tt + 1) * 128],
                                            identR[:])
                        nc.vector.tensor_tensor(
                            out=x2_tiles[t][:, f * 128:(f + 1) * 128],
                            in0=pt[:, tt, :],
                            in1=x_tiles[t][:, f * 128:(f + 1) * 128],
                            op=ALU.add)

                def d_finish(t):
                    x2_t = x2_tiles[t]
                    x_tiles.pop(t)
                    xn2_r = workD.tile([128, C], F32R, tag="xn2_r",
                                       bufs=1)
                    layernorm(x2_t[:], xn2_r[:])
                    for half in range(2):
                        pt = pstD.tile([128, 4, 128], F32R, tag="pst")
                        for j in range(4):
                            c = half * 4 + j
                            nc.tensor.transpose(
                                pt[:, j, :], xn2_r[:, c * 128:(c + 1) * 128],
                                identR[:])
                        nc.scalar.activation(
                            out=xn2T[:, half * 4:(half + 1) * 4,
                                     t * 128:(t + 1) * 128],
                            in_=pt[:], func=AF.Identity,
                            bias=0.0, scale=1.0)

                def e_unit(fb, fs, ch):
                    f = fb * 2 + fs
                    ps = psD.tile([128, 512], F32, tag="ps")
                    for ko in range(NKO):
                        nc.tensor.matmul(
                            ps[:], w1_tiles[fb][:, ko, fs * 128:(fs + 1) * 128],
                            xn2T[:, ko, ch * 512:(ch + 1) * 512],
                            start=(ko == 0), stop=(ko == NKO - 1))
                    nc.scalar.activation(out=h1T[:, f, ch * 512:(ch + 1) * 512],
                                         in_=ps[:], func=AF.Relu,
                                         bias=b1_s[:, f:f + 1], scale=1.0)

                def f_unit(f, ch, finish=False):
                    load_w2(f + 2)
                    halves = (0, 1) if finish else (None,)
                    for hv in halves:
                        lo = 0 if hv in (None, 0) else 256
                        hi = 512 if hv in (None, 1) else 256
                        ps = psD.tile([128, 512], F32, tag="ps", name="ps")
                        for ko in range(DFF // 128):
                            nc.tensor.matmul(
                                ps[:, 0:hi - lo], w2_tiles[f][:, ko, :],
                                h1T[:, ko, ch * 512 + lo:ch * 512 + hi],
                                start=(ko == 0),
                                stop=(ko == DFF // 128 - 1))
                        s = workD.tile([128, 512], F32R, tag="sfc", bufs=3,
                                       name="s")
                        nc.scalar.activation(out=s[:, 0:hi - lo],
                                             in_=ps[:, 0:hi - lo],
                                             func=AF.Identity,
                                             bias=b2_s[:, f:f + 1], scale=1.0)
                        pt = pstD.tile([128, 4, 128], F32R, tag="pst",
                                       name="pt")
                        for tt in range((hi - lo) // 128):
                            t = ch * 4 + (lo // 128) + tt
                            nc.tensor.transpose(
                                pt[:, tt, :],
                                s[:, tt * 128:(tt + 1) * 128], identR[:])
                            nc.vector.tensor_tensor(
                                out=x2_tiles[t][:, f * 128:(f + 1) * 128],
                                in0=pt[:, tt, :],
                                in1=x2_tiles[t][:, f * 128:(f + 1) * 128],
                                op=ALU.add)
                            if finish:
                                f_finish(t)

                def f_finish(t):
                    out_t = x2_tiles.pop(t)
                    eng = nc.gpsimd if t % 2 == 0 else nc.sync
                    eng.dma_start(out=out_d[t * 128:(t + 1) * 128, :],
                                  in_=out_t[:])

                # ---- Stage D: Wo + residual + LN2 ----
                for f in range(NKO):
                    wo_unit(f, 0)
                # ch1 Wo interleaved with ch0 LN2 finish; d_finish(t) must
                # precede wo_unit(t, 1) so the x2w buffer-rotation WAR sems
                # point backwards in each engine's stream (no deadlock)
                for f in range(NKO):
                    if f < 4:
                        d_finish(f)
                    wo_unit(f, 1)
                # ---- Stage E: phase 1 = fb0-3 ch0 only, interleaved with
                # the ch1 LN2 finishes; phase 2 = fb0-3 ch1; phase 3 = rest.
                # (w1p bufs=6 keeps fb0-3 alive until phase 2.) ----
                for fb in range(4):
                    load_w1(fb + 2)
                    e_unit(fb, 0, 0)
                    e_unit(fb, 1, 0)
                    d_finish(4 + fb)
                for fb in range(4):
                    load_w1(6 + fb)
                    e_unit(fb, 0, 1)
                    e_unit(fb, 1, 1)
                for fb in range(4, DFF // 256):
                    load_w1(fb + 2)
                    for fs in range(2):
                        for ch in range(2):
                            e_unit(fb, fs, ch)
                load_w2(0)
                load_w2(1)
                # ---- Stage F: f-outer, W2 loaded once, residual in place
                for f in range(NKO):
                    f_unit(f, 0, finish=(f == NKO - 1))
                    f_unit(f, 1, finish=(f == NKO - 1))

    nc.finalize()
    _legalize_sem_waits(nc)
    return nc


_NC_CACHE = None


def _get_nc():
    global _NC_CACHE
    if _NC_CACHE is None:
        _NC_CACHE = _build_nc()
    return _NC_CACHE


def _shard_inputs(inputs):
    x = np.asarray(inputs["x"], np.float32)
    # Fold the LN affine (g, b) into the following linear layers:
    #   (z*g + b) @ W + c  ==  z @ (diag(g) W) + (c + b @ W)
    g1 = np.asarray(inputs["ln1_g"], np.float32).reshape(C)
    b1n = np.asarray(inputs["ln1_b"], np.float32).reshape(C)
    g2 = np.asarray(inputs["ln2_g"], np.float32).reshape(C)
    b2n = np.asarray(inputs["ln2_b"], np.float32).reshape(C)
    wq = np.ascontiguousarray(
        np.transpose(np.asarray(inputs["Wq"], np.float32), (1, 0, 2)).reshape(C, C))
    wk = np.ascontiguousarray(
        np.transpose(np.asarray(inputs["Wk"], np.float32), (1, 0, 2)).reshape(C, C))
    wv = np.ascontiguousarray(
        np.transpose(np.asarray(inputs["Wv"], np.float32), (1, 0, 2)).reshape(C, C))
    bq = np.asarray(inputs["bq"], np.float32).reshape(C) + b1n @ wq
    bk = np.asarray(inputs["bk"], np.float32).reshape(C) + b1n @ wk
    bv = np.asarray(inputs["bv"], np.float32).reshape(C) + b1n @ wv
    wq = np.ascontiguousarray(wq * g1[:, None])
    wk = np.ascontiguousarray(wk * g1[:, None])
    wv = np.ascontiguousarray(wv * g1[:, None])
    w1f = np.asarray(inputs["W1"], np.float32)
    b1 = np.asarray(inputs["b1"], np.float32).reshape(DFF) + b2n @ w1f
    w1 = (w1f * g2[:, None]).astype(ml_dtypes.bfloat16)
    wo = np.asarray(inputs["Wo"], np.float32).astype(ml_dtypes.bfloat16)
    w2 = np.asarray(inputs["W2"], np.float32).astype(ml_dtypes.bfloat16)
    shared = {
        "wq": wq, "wk": wk, "wv": wv, "wo": wo, "w1": w1, "w2": w2,
        "bq": bq, "bk": bk, "bv": bv,
        "bo": np.asarray(inputs["bo"], np.float32).reshape(C),
        "b1": b1,
        "b2": np.asarray(inputs["b2"], np.float32).reshape(C),
    }
    in_maps = []
    for c in range(N_CORES):
        b, half = c // 2, c % 2
        own = x[b, half * TQ:(half + 1) * TQ]
        other = x[b, (1 - half) * TQ:(2 - half) * TQ]
        x_perm = np.ascontiguousarray(np.concatenate([own, other], axis=0))
        in_maps.append(dict(shared, x=x_perm))
    return in_maps


def _run(inputs, **spmd_kwargs):
    nc = _get_nc()
    in_maps = _shard_inputs(inputs)
    res = run_bass_kernel_spmd(nc, in_maps, core_ids=list(range(N_CORES)),
                               **spmd_kwargs)
    out = np.empty((B, T, C), np.float32)
    for c in range(N_CORES):
        b, half = c // 2, c % 2
        out[b, half * TQ:(half + 1) * TQ] = res.results[c]["out"]
    return out, res


def kernel(**inputs) -> np.ndarray:
    out, _ = _run(inputs)
    return out


# revision 5
# speedup vs baseline: 1.0219x; 1.0002x over previous
"""Trainium2 Bass kernel for a pre-norm transformer block (MHSA + FFN), v2.

Sharding: 8 cores, data parallel over (batch, seq-half). Core c handles
batch c//2, sequence half c%2; K/V run over the full 2048 tokens
(softmax is permutation invariant, own tokens permuted first).

v2 vs v1:
- LN1 tiles interleave with pair-0 QKV so the PE starts early.
- Attention: ACT does ONLY exp, batched N=1024 (one activation per two
  score psum banks), probs written bf16; Q/K/V psum drains on DVE.
- PV keeps the fused ones-column denominator; drained via DVE
  reciprocal + DMA partition-broadcast + DVE multiply (no per-head
  transposes). Odd heads reach partitions 64-127 via SBUF->SBUF DMA.
- oT stays in SBUF (bf16); Wo/W1 bf16 paired with bf16 rhs.
- Stage D residual/LN2 interleaves with Wo ch1 / FFN-up; stage F
  re-streams W2 per 512-token chunk so chunk-0 epilogue overlaps
  chunk-1 matmuls.
- PSUM: attention = sc(4 banks) + qkv(2) + pv(2); D/E/F = ps(4)+pst(4).
"""
import contextlib

import numpy as np
import ml_dtypes

import concourse.bass as bass
import concourse.tile as tile
import concourse.mybir as mybir
from concourse.bass_utils import run_bass_kernel_spmd
from concourse.masks import make_identity

B, T, C = 4, 2048, 1024
H, DH = 16, 64
DFF = 4 * C
N_CORES = 8
TQ = T // 2
TS = T
NKO = C // 128
F32R = mybir.dt.float32r
F32 = mybir.dt.float32
BF16 = mybir.dt.bfloat16
EXP_BIAS = -128.0
EPS = 1e-5
AF = mybir.ActivationFunctionType
ALU = mybir.AluOpType

_ev_counter = [0]


def _legalize_sem_waits(nc):
    """This walrus build accepts at most 1 sem-wait per regular
    instruction (2 per InstEventSemaphore); split the rest ourselves."""
    for func in nc.m.functions:
        for bb in func.blocks:
            new = []
            changed = False
            for inst in bb.instructions:
                si = inst.sync_info
                cap = 2 if isinstance(inst, mybir.InstEventSemaphore) else 1
                if si is not None and len(si.on_wait) > cap:
                    waits = list(si.on_wait)
                    for i in range(cap, len(waits), 2):
                        _ev_counter[0] += 1
                        e = mybir.InstEventSemaphore(
                            name=f"EVSPLIT-{_ev_counter[0]}", ins=[], outs=[])
                        e.engine = inst.engine
                        e.sync_info = mybir.SyncInfo(
                            on_wait=waits[i:i + 2], on_update=[])
                        new.append(e)
                    inst.sync_info = mybir.SyncInfo(
                        on_wait=waits[:cap], on_update=list(si.on_update))
                    changed = True
                new.append(inst)
            if changed:
                bb.instructions = new


def _bcast_row(ap, p):
    """Broadcast a [1, n] AP across p partitions (DMA source)."""
    return bass.AP(tensor=ap.tensor, offset=ap.offset,
                   ap=[[0, p]] + [list(x) for x in ap.ap[1:]])


def _bcast_dram(ap, p=128):
    return bass.AP(tensor=ap.tensor, offset=ap.offset,
                   ap=[[0, p]] + [list(x) for x in ap.ap])


def _build_nc():
    nc = bass.Bass()

    x_d = nc.dram_tensor("x", [T, C], F32, kind="ExternalInput")
    wq_d = nc.dram_tensor("wq", [C, C], F32R, kind="ExternalInput")
    wk_d = nc.dram_tensor("wk", [C, C], F32R, kind="ExternalInput")
    wv_d = nc.dram_tensor("wv", [C, C], F32R, kind="ExternalInput")
    wo_d = nc.dram_tensor("wo", [C, C], BF16, kind="ExternalInput")
    w1_d = nc.dram_tensor("w1", [C, DFF], BF16, kind="ExternalInput")
    w2_d = nc.dram_tensor("w2", [DFF, C], BF16, kind="ExternalInput")
    bq_d = nc.dram_tensor("bq", [C], F32, kind="ExternalInput")
    bk_d = nc.dram_tensor("bk", [C], F32, kind="ExternalInput")
    bv_d = nc.dram_tensor("bv", [C], F32, kind="ExternalInput")
    bo_d = nc.dram_tensor("bo", [C], F32, kind="ExternalInput")
    b1_d = nc.dram_tensor("b1", [DFF], F32, kind="ExternalInput")
    b2_d = nc.dram_tensor("b2", [C], F32, kind="ExternalInput")
    out_d = nc.dram_tensor("out", [TQ, C], F32, kind="ExternalOutput")

    dr_h = nc.dram_tensor("dr_h", [H, 2, 512], F32)

    wq_r = wq_d.rearrange("(o p) f -> p o f", p=128)
    wk_r = wk_d.rearrange("(o p) f -> p o f", p=128)
    wv_r = wv_d.rearrange("(o p) f -> p o f", p=128)
    wo_r = wo_d.rearrange("(o p) f -> p o f", p=128)
    w1_r = w1_d.rearrange("(o p) f -> p o f", p=128)
    w2_r = w2_d.rearrange("(o p) f -> p o f", p=128)

    with tile.TileContext(nc) as tc:
        with contextlib.ExitStack() as top:
            consts = top.enter_context(tc.tile_pool(name="consts", bufs=1))
            stats = top.enter_context(tc.tile_pool(name="stats", bufs=8))

            identF = consts.tile([128, 128], F32, tag="identf")
            make_identity(nc, identF)
            identR = consts.tile([128, 128], F32R, tag="identr")
            nc.vector.tensor_copy(out=identR[:], in_=identF[:])
            ebias = consts.tile([128, 1], F32, tag="ebias")
            nc.vector.memset(ebias[:], EXP_BIAS)
            eps_t = consts.tile([128, 1], F32, tag="eps")
            nc.vector.memset(eps_t[:], EPS)
            bq_s = consts.tile([128, NKO], F32, tag="bq")
            bk_s = consts.tile([128, NKO], F32, tag="bk")
            bo_s = consts.tile([128, NKO], F32, tag="bo")
            b2_s = consts.tile([128, NKO], F32, tag="b2")
            b1_s = consts.tile([128, DFF // 128], F32, tag="b1")
            for dst, src in ((bq_s, bq_d), (bk_s, bk_d), (bo_s, bo_d),
                             (b2_s, b2_d), (b1_s, b1_d)):
                nc.sync.dma_start(out=dst[:],
                                  in_=src.rearrange("(o p) -> p o", p=128))
            bv_r = consts.tile([128, C], F32, tag="bvr")
            nc.gpsimd.dma_start(out=bv_r[:], in_=_bcast_dram(bv_d[:]))

            def layernorm(x_ap, out_ap, apply_eng=None):
                st = stats.tile([128, 2, 6], F32, tag="bnstats")
                mv = stats.tile([128, 2], F32, tag="bnaggr")
                xg = x_ap.rearrange("p (s d) -> p s d", s=2)
                for s in range(2):
                    nc.vector.bn_stats(out=st[:, s, :], in_=xg[:, s, :])
                nc.vector.bn_aggr(out=mv[:], in_=st[:])
                rstd = stats.tile([128, 1], F32, tag="rstd")
                nc.scalar.activation(out=rstd[:], in_=mv[:, 1:2],
                                     func=AF.Sqrt, bias=eps_t[:], scale=1.0)
                nc.vector.reciprocal(out=rstd[:], in_=rstd[:])
                (apply_eng or nc.vector).tensor_scalar(
                    out=out_ap, in0=x_ap,
                    scalar1=mv[:, 0:1], scalar2=rstd[:],
                    op0=ALU.subtract, op1=ALU.mult)

            # oT outlives the attention scope (consumed by Wo)
            oTp = top.enter_context(tc.tile_pool(name="oTp", bufs=1))
            oT = oTp.tile([128, NKO, TQ], BF16, tag="oT")

            # =========== Attention phase ===========
            with contextlib.ExitStack() as abc:
                qkp = abc.enter_context(tc.tile_pool(name="qkp", bufs=2))
                vgp = abc.enter_context(tc.tile_pool(name="vgp", bufs=2))
                prb = abc.enter_context(tc.tile_pool(name="prb", bufs=2))
                drp = abc.enter_context(tc.tile_pool(name="drp", bufs=2))
                scp = abc.enter_context(
                    tc.tile_pool(name="scp", bufs=2, space="PSUM"))
                qkvp = abc.enter_context(
                    tc.tile_pool(name="qkvp", bufs=2, space="PSUM"))
                pvp = abc.enter_context(
                    tc.tile_pool(name="pvp", bufs=2, space="PSUM"))

                qk_tiles = {}
                vg_tiles = {}

                qkvscope = contextlib.ExitStack()
                if True:
                    xnp = qkvscope.enter_context(
                        tc.tile_pool(name="xnp", bufs=1))
                    xnT = xnp.tile([128, NKO, T], F32R, tag="xnT")
                    workA = qkvscope.enter_context(
                        tc.tile_pool(name="workA", bufs=3))
                    wgp = qkvscope.enter_context(
                        tc.tile_pool(name="wgp", bufs=2))

                    def a_unit(t):
                        x_t = workA.tile([128, C], F32, tag="x_t")
                        if t < 4:
                            with tc.high_priority():
                                nc.scalar.dma_start(
                                    out=x_t[0:64, :],
                                    in_=x_d[t * 128:t * 128 + 64, :])
                                nc.sync.dma_start(
                                    out=x_t[64:128, :],
                                    in_=x_d[t * 128 + 64:(t + 1) * 128, :])
                        else:
                            nc.scalar.dma_start(
                                out=x_t[:],
                                in_=x_d[t * 128:(t + 1) * 128, :])
                        xn_r = workA.tile([128, C], F32R, tag="xn_r")
                        layernorm(x_t[:], xn_r[:])
                        pt = scp.tile([128, NKO, 128], F32R, tag="sc",
                                      name="pt")
                        for c in range(NKO):
                            nc.tensor.transpose(
                                pt[:, c, :],
                                xn_r[:, c * 128:(c + 1) * 128], identR[:])
                        nc.scalar.activation(
                            out=xnT[:, :, t * 128:(t + 1) * 128],
                            in_=pt[:], func=AF.Identity,
                            bias=0.0, scale=1.0)

                    def qk_gen(p):
                        wqt = wgp.tile([128, NKO, 128], F32R, tag="wqt",
                                       bufs=1)
                        wkt = wgp.tile([128, NKO, 128], F32R, tag="wkt",
                                       bufs=1)
                        nc.sync.dma_start(
                            out=wqt[:], in_=wq_r[:, :, p * 128:(p + 1) * 128])
                        nc.sync.dma_start(
                            out=wkt[:], in_=wk_r[:, :, p * 128:(p + 1) * 128])
                        qp = qkp.tile([128, TQ], F32R, tag="qp")
                        kp = qkp.tile([128, TS], F32R, tag="kp")
                        qk_tiles[p] = (qp, kp)
                        yield

                        def qu(ch):
                            pq = qkvp.tile([128, 512], F32, tag="qkv",
                                           name="pq")
                            for ko in range(NKO):
                                nc.tensor.matmul(
                                    pq[:], wqt[:, ko, :],
                                    xnT[:, ko, ch * 512:(ch + 1) * 512],
                                    start=(ko == 0), stop=(ko == NKO - 1))
                            nc.vector.tensor_scalar_add(
                                out=qp[:, ch * 512:(ch + 1) * 512],
                                in0=pq[:], scalar1=bq_s[:, p:p + 1])

                        def ku(ch):
                            pk = qkvp.tile([128, 512], F32, tag="qkv",
                                           name="pk")
                            for ko in range(NKO):
                                nc.tensor.matmul(
                                    pk[:], wkt[:, ko, :],
                                    xnT[:, ko, ch * 512:(ch + 1) * 512],
                                    start=(ko == 0), stop=(ko == NKO - 1))
                            nc.vector.tensor_scalar_add(
                                out=kp[:, ch * 512:(ch + 1) * 512],
                                in0=pk[:], scalar1=bk_s[:, p:p + 1])

                        for kind, ch in (("q", 0), ("k", 0), ("q", 1),
                                         ("k", 1), ("k", 2), ("k", 3)):
                            qu(ch) if kind == "q" else ku(ch)
                            yield

                    def v_gen(g):
                        wvt = wgp.tile([128, NKO, 256], F32R, tag="wvt",
                                       bufs=1)
                        nc.sync.dma_start(
                            out=wvt[:], in_=wv_r[:, :, g * 256:(g + 1) * 256])
                        vg = vgp.tile([128, TS // 128, 4, 65], BF16, tag="vg")
                        vg_tiles[g] = vg
                        nc.vector.memset(vg[:, :, :, 64:65], 1.0)
                        yield
                        for j in range(TS // 256):
                            pvv = qkvp.tile([128, 512], F32, tag="qkv")
                            for i in range(2):
                                to = 2 * j + i
                                for ko in range(NKO):
                                    nc.tensor.matmul(
                                        pvv[0:128, i * 256:(i + 1) * 256],
                                        xnT[:, ko, to * 128:(to + 1) * 128],
                                        wvt[:, ko, :],
                                        start=(ko == 0), stop=(ko == NKO - 1))
                            for i in range(2):
                                to = 2 * j + i
                                nc.vector.tensor_tensor(
                                    out=vg[:, to, :, 0:64],
                                    in0=pvv[:, i * 256:(i + 1) * 256].rearrange(
                                        "p (h d) -> p h d", d=64),
                                    in1=bv_r[:, g * 256:(g + 1) * 256].rearrange(
                                        "p (h d) -> p h d", d=64),
                                    op=ALU.add)
                            yield

                    # ---- prologue: LN1 tiles interleaved with pair-0 QKV
                    q0 = qk_gen(0)
                    next(q0)                    # wq/wk DMAs in flight
                    v0 = v_gen(0)
                    next(v0)                    # wv DMA + ones memset
                    for t in range(4):
                        a_unit(t)
                    next(q0)                    # Q(0,0)
                    next(q0)                    # K(0,0)
                    for t in range(4, 8):
                        a_unit(t)
                    next(q0)                    # Q(0,1)
                    next(q0)                    # K(0,1)
                    next(v0)                    # V(0,0)
                    next(v0)                    # V(0,1)
                    for t in range(8, 12):
                        a_unit(t)
                    next(q0)                    # K(0,2)
                    next(v0)                    # V(0,2)
                    next(v0)                    # V(0,3)
                    for t in range(12, 16):
                        a_unit(t)
                    next(q0)                    # K(0,3)
                    for _ in v0:
                        pass
                    for _ in q0:
                        pass

                    # ---- pair loop with QKV pulled between attention units
                    pull_src = {
                        0: [qk_gen(1)],
                        1: [qk_gen(2), v_gen(1)],
                        2: [qk_gen(3)],
                        3: [qk_gen(4), v_gen(2)],
                        4: [qk_gen(5)],
                        5: [qk_gen(6), v_gen(3)],
                        6: [qk_gen(7)],
                        7: [],
                    }

                    def attention_hc(h, ch, pull, pending):
                        p = h // 2
                        base = (h % 2) * 64
                        hl = h % 4
                        qp, kp = qk_tiles[p]
                        vg = vg_tiles[h // 4]
                        probsT = prb.tile([128, TS // 128, 512], BF16,
                                          tag="probs")
                        pvt = pvp.tile([128, 512], F32, tag="pv")

                        def s_unit(g):
                            sct = scp.tile([128, 1024], F32, tag="sc")
                            for i in range(2):
                                tso = 2 * g + i
                                nc.tensor.matmul(
                                    sct[:, i * 512:(i + 1) * 512],
                                    kp[base:base + DH,
                                       tso * 128:(tso + 1) * 128],
                                    qp[base:base + DH,
                                       ch * 512:(ch + 1) * 512],
                                    start=True, stop=True)
                            nc.scalar.activation(
                                out=probsT[:, 2 * g:2 * g + 2, :], in_=sct[:],
                                func=AF.Exp, scale=8.0, bias=ebias[:])

                        def p_unit(g):
                            for i in range(2):
                                to = 2 * g + i
                                nc.tensor.matmul(
                                    pvt[0:DH + 1, :],
                                    vg[:, to, hl, 0:DH + 1],
                                    probsT[:, to, :],
                                    start=(to == 0), stop=(to == 15))

                        s_unit(0)
                        s_unit(1)
                        if pending:
                            pending[0]()
                            pending.clear()
                        for g in range(6):
                            pull()
                            p_unit(g)
                            s_unit(g + 2)
                        p_unit(6)
                        p_unit(7)
                        pull()

                        def drain():
                        # denominator: reciprocal of psum row 64 (stays on
                        # partition 64), then DMA-broadcast to partitions
                        # 0-63 (compute engines cannot shift partitions)
                            rc = drp.tile([128, 512], F32, tag="rbc",
                                          name="rc")
                            nc.vector.reciprocal(out=rc[64:65, :],
                                                 in_=pvt[64:65, :])
                            nc.gpsimd.dma_start(out=dr_h[h, ch, :],
                                                in_=rc[64:65, :])
                            nc.gpsimd.dma_start(
                                out=rc[0:64, :],
                                in_=_bcast_dram(dr_h[h, ch, :], 64))
                            rb = rc
                            if h % 2 == 0:
                                nc.vector.tensor_tensor(
                                    out=oT[0:64, p,
                                           ch * 512:(ch + 1) * 512],
                                    in0=pvt[0:64, :], in1=rb[0:64, :],
                                    op=ALU.mult)
                            else:
                                so = drp.tile([64, 512], BF16, tag="so",
                                              name="so")
                                nc.vector.tensor_tensor(
                                    out=so[0:64, :], in0=pvt[0:64, :],
                                    in1=rb[0:64, :], op=ALU.mult)
                                nc.gpsimd.dma_start(
                                    out=oT[64:128, p,
                                           ch * 512:(ch + 1) * 512],
                                    in_=so[0:64, :])
                        pending.append(drain)

                    pending_drain = []
                    wo_tiles = {}
                    w1_tiles = {}
                    x_tiles = {}
                    x2_tiles = {}
                    for p in range(8):
                        gens = pull_src[p]

                        def pull():
                            while gens:
                                try:
                                    next(gens[0])
                                    return
                                except StopIteration:
                                    gens.pop(0)

                        for h in (2 * p, 2 * p + 1):
                            for ch in range(2):
                                attention_hc(h, ch, pull, pending_drain)
                        # generators must drain within their pair so the
                        # xnT/wgp scope can close after pair 6
                        if p >= 5:
                            pull()
                            while gens:
                                pull()
                        if p == 6:
                            # xnT/workA/wgp are dead: free them and pull
                            # the stage-D working set in under pair 7
                            qkvscope.close()
                            wop = top.enter_context(
                                tc.tile_pool(name="wop", bufs=2,
                                             side="right"))
                            w1p = top.enter_context(
                                tc.tile_pool(name="w1p", bufs=6,
                                             side="right"))
                            x2w = top.enter_context(
                                tc.tile_pool(name="x2w", bufs=4,
                                             side="right"))

                            def load_wo(k):
                                if k >= 16 or k in wo_tiles:
                                    return
                                f = k % NKO
                                wot = wop.tile([128, NKO, 128], BF16,
                                               tag="wo", name="wot")
                                nc.sync.dma_start(
                                    out=wot[:],
                                    in_=wo_r[:, :, f * 128:(f + 1) * 128])
                                wo_tiles[k] = wot

                            def load_w1(fb):
                                if fb >= DFF // 256 or fb in w1_tiles:
                                    return
                                w1t = w1p.tile([128, NKO, 256], BF16,
                                               tag="w1t", name="w1t")
                                nc.sync.dma_start(
                                    out=w1t[:],
                                    in_=w1_r[:, :, fb * 256:(fb + 1) * 256])
                                w1_tiles[fb] = w1t

                            def load_x(t):
                                x_t = x2w.tile([128, C], F32, tag="x_t",
                                               name="x_t")
                                nc.scalar.dma_start(
                                    out=x_t[:],
                                    in_=x_d[t * 128:(t + 1) * 128, :])
                                x_tiles[t] = x_t
                                x2_tiles[t] = x2w.tile(
                                    [128, C], F32, tag="x2_t",
                                    name="x2_t", bufs=8)

                            for k in range(4):
                                load_wo(k)
                            load_w1(0)
                            load_w1(1)
                            for t in range(8):
                                load_x(t)
                    if pending_drain:
                        pending_drain[0]()
                        pending_drain.clear()

            # =========== Stages D/E/F ===========
            with contextlib.ExitStack() as dstk:
                w2p = dstk.enter_context(tc.tile_pool(name="w2p", bufs=2))
                fm = dstk.enter_context(tc.tile_pool(name="fm", bufs=1))
                workD = dstk.enter_context(tc.tile_pool(name="workD", bufs=2))
                psD = dstk.enter_context(
                    tc.tile_pool(name="psD", bufs=4, space="PSUM"))
                pstD = dstk.enter_context(
                    tc.tile_pool(name="pstD", bufs=4, space="PSUM"))

                w2_tiles = {}

                def load_w2(f):
                    if f >= NKO or f in w2_tiles:
                        return
                    w2t = w2p.tile([128, DFF // 128, 128], BF16, tag="w2t")
                    nc.sync.dma_start(
                        out=w2t[:], in_=w2_r[:, :, f * 128:(f + 1) * 128])
                    w2_tiles[f] = w2t

                xn2T = fm.tile([128, NKO, TQ], BF16, tag="xn2T")
                h1T = fm.tile([128, DFF // 128, TQ], BF16, tag="h1T")

                def wo_unit(f, ch):
                    k = ch * NKO + f
                    load_wo(k + 2)
                    ps = psD.tile([128, 512], F32, tag="ps")
                    for ko in range(NKO):
                        nc.tensor.matmul(
                            ps[:], wo_tiles[k][:, ko, :],
                            oT[:, ko, ch * 512:(ch + 1) * 512],
                            start=(ko == 0), stop=(ko == NKO - 1))
                    s = workD.tile([128, 512], F32R, tag="sfc", bufs=3)
                    nc.scalar.activation(out=s[:], in_=ps[:],
                                         func=AF.Identity,
                                         bias=bo_s[:, f:f + 1], scale=1.0)
                    pt = pstD.tile([128, 4, 128], F32R, tag="pst")
                    for tt in range(4):
                        t = ch * 4 + tt
                        nc.tensor.transpose(pt[:, tt, :],
                                            s[:, tt * 128:(tt + 1) * 128],
                                            identR[:])
                        nc.vector.tensor_tensor(
                            out=x2_tiles[t][:, f * 128:(f + 1) * 128],
                            in0=pt[:, tt, :],
                            in1=x_tiles[t][:, f * 128:(f + 1) * 128],
                            op=ALU.add)

                def d_finish(t):
                    x2_t = x2_tiles[t]
                    x_tiles.pop(t)
                    xn2_r = workD.tile([128, C], F32R, tag="xn2_r",
                                       bufs=1)
                    layernorm(x2_t[:], xn2_r[:])
                    for half in range(2):
                        pt = pstD.tile([128, 4, 128], F32R, tag="pst")
                        for j in range(4):
                            c = half * 4 + j
                            nc.tensor.transpose(
                                pt[:, j, :], xn2_r[:, c * 128:(c + 1) * 128],
                                identR[:])
                        nc.scalar.activation(
                            out=xn2T[:, half * 4:(half + 1) * 4,
                                     t * 128:(t + 1) * 128],
                            in_=pt[:], func=AF.Identity,
                            bias=0.0, scale=1.0)

                def e_unit(fb, fs, ch):
                    f = fb * 2 + fs
                    ps = psD.tile([128, 512], F32, tag="ps")
                    for ko in range(NKO):
                        nc.tensor.matmul(
                            ps[:], w1_tiles[fb][:, ko, fs * 128:(fs + 1) * 128],
                            xn2T[:, ko, ch * 512:(ch + 1) * 512],
                            start=(ko == 0), stop=(ko == NKO - 1))
                    nc.scalar.activation(out=h1T[:, f, ch * 512:(ch + 1) * 512],
                                         in_=ps[:], func=AF.Relu,
                                         bias=b1_s[:, f:f + 1], scale=1.0)

                def f_unit(f, ch, finish=False):
                    load_w2(f + 2)
                    halves = (0, 1) if finish else (None,)
                    for hv in halves:
                        lo = 0 if hv in (None, 0) else 256
                        hi = 512 if hv in (None, 1) else 256
                        ps = psD.tile([128, 512], F32, tag="ps", name="ps")
                        for ko in range(DFF // 128):
                            nc.tensor.matmul(
                                ps[:, 0:hi - lo], w2_tiles[f][:, ko, :],
                                h1T[:, ko, ch * 512 + lo:ch * 512 + hi],
                                start=(ko == 0),
                                stop=(ko == DFF // 128 - 1))
                        s = workD.tile([128, 512], F32R, tag="sfc", bufs=3,
                                       name="s")
                        nc.scalar.activation(out=s[:, 0:hi - lo],
                                             in_=ps[:, 0:hi - lo],
                                             func=AF.Identity,
                                             bias=b2_s[:, f:f + 1], scale=1.0)
                        pt = pstD.tile([128, 4, 128], F32R, tag="pst",
                                       name="pt")
                        for tt in range((hi - lo) // 128):
                            t = ch * 4 + (lo // 128) + tt
                            nc.tensor.transpose(
                                pt[:, tt, :],
                                s[:, tt * 128:(tt + 1) * 128], identR[:])
                            nc.vector.tensor_tensor(
                                out=x2_tiles[t][:, f * 128:(f + 1) * 128],
                                in0=pt[:, tt, :],
                                in1=x2_tiles[t][:, f * 128:(f + 1) * 128],
                                op=ALU.add)
                            if finish:
                                f_finish(t)

                def f_finish(t):
                    out_t = x2_tiles.pop(t)
                    eng = nc.gpsimd if t % 2 == 0 else nc.sync
                    eng.dma_start(out=out_d[t * 128:(t + 1) * 128, :],
                                  in_=out_t[:])

                # ---- Stage D: Wo + residual + LN2 ----
                for f in range(NKO):
                    wo_unit(f, 0)
                # ch1 Wo interleaved with ch0 LN2 finish; d_finish(t) must
                # precede wo_unit(t, 1) so the x2w buffer-rotation WAR sems
                # point backwards in each engine's stream (no deadlock)
                for f in range(NKO):
                    if f < 4:
                        d_finish(f)
                    wo_unit(f, 1)
                # ---- Stage E: phase 1 = fb0-3 ch0 only, interleaved with
                # the ch1 LN2 finishes; phase 2 = fb0-3 ch1; phase 3 = rest.
                # (w1p bufs=6 keeps fb0-3 alive until phase 2.) ----
                for fb in range(4):
                    load_w1(fb + 2)
                    e_unit(fb, 0, 0)
                    e_unit(fb, 1, 0)
                    d_finish(4 + fb)
                for fb in range(4):
                    load_w1(6 + fb)
                    e_unit(fb, 0, 1)
                    e_unit(fb, 1, 1)
                for fb in range(4, DFF // 256):
                    load_w1(fb + 2)
                    for fs in range(2):
                        for ch in range(2):
                            e_unit(fb, fs, ch)
                load_w2(0)
                load_w2(1)
                # ---- Stage F: f-outer, W2 loaded once, residual in place
                for f in range(NKO):
                    f_unit(f, 0, finish=(f == NKO - 1))
                    f_unit(f, 1, finish=(f == NKO - 1))

    nc.finalize()
    _legalize_sem_waits(nc)
    return nc


_NC_CACHE = None


def _get_nc():
    global _NC_CACHE
    if _NC_CACHE is None:
        _NC_CACHE = _build_nc()
    return _NC_CACHE


def _shard_inputs(inputs):
    x = np.asarray(inputs["x"], np.float32)
    # Fold the LN affine (g, b) into the following linear layers:
    #   (z*g + b) @ W + c  ==  z @ (diag(g) W) + (c + b @ W)
    g1 = np.asarray(inputs["ln1_g"], np.float32).reshape(C)
    b1n = np.asarray(inputs["ln1_b"], np.float32).reshape(C)
    g2 = np.asarray(inputs["ln2_g"], np.float32).reshape(C)
    b2n = np.asarray(inputs["ln2_b"], np.float32).reshape(C)
    wq = np.ascontiguousarray(
        np.transpose(np.asarray(inputs["Wq"], np.float32), (1, 0, 2)).reshape(C, C))
    wk = np.ascontiguousarray(
        np.transpose(np.asarray(inputs["Wk"], np.float32), (1, 0, 2)).reshape(C, C))
    wv = np.ascontiguousarray(
        np.transpose(np.asarray(inputs["Wv"], np.float32), (1, 0, 2)).reshape(C, C))
    bq = np.asarray(inputs["bq"], np.float32).reshape(C) + b1n @ wq
    bk = np.asarray(inputs["bk"], np.float32).reshape(C) + b1n @ wk
    bv = np.asarray(inputs["bv"], np.float32).reshape(C) + b1n @ wv
    wq = np.ascontiguousarray(wq * g1[:, None])
    wk = np.ascontiguousarray(wk * g1[:, None])
    wv = np.ascontiguousarray(wv * g1[:, None])
    w1f = np.asarray(inputs["W1"], np.float32)
    b1 = np.asarray(inputs["b1"], np.float32).reshape(DFF) + b2n @ w1f
    w1 = (w1f * g2[:, None]).astype(ml_dtypes.bfloat16)
    wo = np.asarray(inputs["Wo"], np.float32).astype(ml_dtypes.bfloat16)
    w2 = np.asarray(inputs["W2"], np.float32).astype(ml_dtypes.bfloat16)
    shared = {
        "wq": wq, "wk": wk, "wv": wv, "wo": wo, "w1": w1, "w2": w2,
        "bq": bq, "bk": bk, "bv": bv,
        "bo": np.asarray(inputs["bo"], np.float32).reshape(C),
        "b1": b1,
        "b2": np.asarray(inputs["b2"], np.float32).reshape(C),
    }
    in_maps = []
    for c in range(N_CORES):
        b, half = c // 2, c % 2
        own = x[b, half * TQ:(half + 1) * TQ]
        other = x[b, (1 - half) * TQ:(2 - half) * TQ]
        x_perm = np.ascontiguousarray(np.concatenate([own, other], axis=0))
        in_maps.append(dict(shared, x=x_perm))
    return in_maps


def _run(inputs, **spmd_kwargs):
    nc = _get_nc()
    in_maps = _shard_inputs(inputs)
    res = run_bass_kernel_spmd(nc, in_maps, core_ids=list(range(N_CORES)),
                               **spmd_kwargs)
    out = np.empty((B, T, C), np.float32)
    for c in range(N_CORES):
        b, half = c // 2, c % 2
        out[b, half * TQ:(half + 1) * TQ] = res.results[c]["out"]
    return out, res


def kernel(**inputs) -> np.ndarray:
    out, _ = _run(inputs)
    return out
